# revision 1
# baseline (speedup 1.0000x reference)
"""DeepGCNLayer (GraphConv norm='both' + BatchNorm + ReLU + residual) on 8 trn2 cores.

Sharding: nodes padded to NPAD=100352, split into 8 ranges (98 node-tiles of
128 per core). Edges routed to the core owning their dst (dst-sorted), then
per (dst-tile, src-window) padded to a uniform K_w chunks of 128 so every
core runs one SPMD program. src-windows of 25088 rows keep dma_gather's int16
indices in range.

Device pipeline per core:
  1. z pass (own slice only): z = (x*ns)@W in bf16 via per-chunk PE
     transpose + matmul, written to cc_zin; ONE AllGather concatenates all
     cores' slices into pair-shared z_all [npad, D] bf16 (global node order),
     whose 4 window row-views feed the gathers (idx mapping unchanged).
  2. Per tile-group x window: one dma_gather (bf16 z rows, 4 SWDGE queues).
     Per tile: one-hot S chunks (iota==eloc, bf16) via DVE, then
     hN[dst,feat] += S^T @ E accumulated in PSUM over ~20 chunks (node-major
     directly - W was pre-applied). Scale by norm_dst = per-partition
     broadcast, store bf16. BN stats per feature via interleaved
     ones^T @ h PSUM-accumulating matmuls -> [1,128] rows.
  3. stats AllReduce ([1,256] row) -> row math -> rank-1 broadcast of
     scale/shift -> phase B: relu(h*sc+tc)+x elementwise (no transposes),
     batched 7-tile loads/stores.
"""

import sys

if "/opt/trn_rl_repo" not in sys.path:
    sys.path.insert(0, "/opt/trn_rl_repo")

import numpy as np

P = 128
D = 128
NCORES = 8
BN_EPS = 1e-5
WIN = 25088          # src-window rows (int16-safe)
GROUP = 7            # dst tiles per gather group
XSC = 7              # node tiles per z-pass iteration
PBC = 7              # node tiles per phase-B load/store batch

_NC_CACHE = {}


def build_program(npad, nt, kws, n_real):
    """kws: tuple of chunks-per-window per dst tile (uniform across tiles)."""
    import concourse.bacc as bacc
    import concourse.tile as tile
    from concourse import mybir
    from concourse.masks import make_identity

    f32 = mybir.dt.float32
    bf16 = mybir.dt.bfloat16
    i32 = mybir.dt.int32
    i16 = mybir.dt.int16
    OP = mybir.AluOpType
    AF = mybir.ActivationFunctionType

    nodes_pc = nt * P
    ktot = sum(kws)
    nwin = len(kws)
    ngroups = nt // GROUP
    # idx16 columns per (group, window) and totals
    cols_gw = [GROUP * kw * P // 16 for kw in kws]
    idx_cols = ngroups * sum(cols_gw)

    nc = bacc.Bacc("TRN2", target_bir_lowering=False, debug=False,
                   num_devices=NCORES, num_swdge_queues=4)

    xown = nc.dram_tensor("xown", [nodes_pc, D], f32, kind="ExternalInput")
    wmat = nc.dram_tensor("wmat", [D, D], f32, kind="ExternalInput")
    grow = nc.dram_tensor("grow", [1, D], f32, kind="ExternalInput")
    brow = nc.dram_tensor("brow", [1, D], f32, kind="ExternalInput")
    dgo = nc.dram_tensor("dgo", [P, nt], i32, kind="ExternalInput")
    dgi = nc.dram_tensor("dgi", [P, nt], i32, kind="ExternalInput")
    idxs = nc.dram_tensor("idxs", [P, idx_cols], i16, kind="ExternalInput")
    eloc = nc.dram_tensor("eloc", [P, nt * ktot], bf16, kind="ExternalInput")
    out = nc.dram_tensor("out", [nodes_pc, D], f32, kind="ExternalOutput")

    half_pc = nodes_pc // 2
    cc_za = nc.dram_tensor("cc_za", [half_pc, D], bf16)
    cc_zb = nc.dram_tensor("cc_zb", [half_pc, D], bf16)
    z_ta = nc.dram_tensor("z_ta", [half_pc * NCORES, D], bf16,
                          addr_space="Shared")
    z_tb = nc.dram_tensor("z_tb", [half_pc * NCORES, D], bf16,
                          addr_space="Shared")

    with tile.TileContext(nc) as tc:
        with (
            tc.tile_pool(name="const", bufs=1) as constp,
            tc.tile_pool(name="norm", bufs=1) as normp,
            tc.tile_pool(name="xst", bufs=2) as xsp,
            tc.tile_pool(name="tr", bufs=3) as trp,
            tc.tile_pool(name="edges", bufs=1) as edgep,
            tc.tile_pool(name="gath", bufs=2) as gathp,
            tc.tile_pool(name="s", bufs=1) as sp,
            tc.tile_pool(name="work", bufs=2) as workp,
            tc.tile_pool(name="stats", bufs=1) as statp,
            tc.tile_pool(name="store", bufs=1) as storep,
            tc.tile_pool(name="io", bufs=2) as iop,
            tc.tile_pool(name="psA", bufs=2, space="PSUM") as psA,
            tc.tile_pool(name="psB", bufs=2, space="PSUM") as psB,
            tc.tile_pool(name="psS", bufs=1, space="PSUM") as psS,
            tc.tile_pool(name="dram", bufs=2, space="DRAM") as dramp,
        ):
            # ---- edge metadata first (scalar queue; gathers need these) ----
            idxs_sb = edgep.tile([P, idx_cols], i16, tag="idxs")
            nc.scalar.dma_start(out=idxs_sb[:], in_=idxs[:])
            eloc_sb = edgep.tile([P, nt * ktot], bf16, tag="eloc")
            nc.scalar.dma_start(out=eloc_sb[:], in_=eloc[:])

            # ---- constants -------------------------------------------------
            iota = constp.tile([P, P], bf16, tag="iota")
            nc.gpsimd.iota(iota[:], pattern=[[1, P]], base=0,
                           channel_multiplier=0,
                           allow_small_or_imprecise_dtypes=True)
            ident = constp.tile([P, P], bf16, tag="ident")
            make_identity(nc, ident[:])
            ones1 = constp.tile([1, P], f32, tag="ones1")
            nc.vector.memset(ones1[:], 1.0)
            ones_c = constp.tile([P, 1], bf16, tag="ones_c")
            nc.vector.memset(ones_c[:], 1.0)
            w_f32 = constp.tile([P, D], f32, tag="wf32")
            nc.sync.dma_start(out=w_f32[:], in_=wmat[:])
            w_sb = constp.tile([P, D], bf16, tag="wsb")
            nc.vector.tensor_copy(w_sb[:], w_f32[:])
            g_row = constp.tile([1, D], f32, tag="grow")
            nc.sync.dma_start(out=g_row[:], in_=grow[:])
            be_row = constp.tile([1, D], f32, tag="berow")
            nc.sync.dma_start(out=be_row[:], in_=brow[:])

            # ---- norm arrays (own range, F-order [P, nt]) ------------------
            deg = normp.tile([P, nt], i32, tag="deg")
            nc.sync.dma_start(out=deg[:], in_=dgo[:])
            degf = normp.tile([P, nt], f32, tag="degf")
            nc.vector.tensor_scalar_max(degf[:], deg[:], 1.0)
            nc.scalar.sqrt(degf[:], degf[:])
            ns_f = constp.tile([P, nt], f32, tag="ns_f")
            nc.vector.reciprocal(ns_f[:], degf[:])

            deg2 = normp.tile([P, nt], i32, tag="deg2")
            nc.sync.dma_start(out=deg2[:], in_=dgi[:])
            deg2f = normp.tile([P, nt], f32, tag="deg2f")
            nc.vector.tensor_scalar_max(deg2f[:], deg2[:], 1.0)
            nc.scalar.sqrt(deg2f[:], deg2f[:])
            nd_f = constp.tile([P, nt], f32, tag="nd_f")
            nc.vector.reciprocal(nd_f[:], deg2f[:])

            # ---- z pass: z = (x*ns)@W -> bf16, own slice only --------------
            # two halves, each followed by its AllGather so window 0/1
            # gathers start while half B is still collecting
            half_t = nt // 2
            for half, (cc_z, z_t_out) in enumerate(
                    [(cc_za, z_ta), (cc_zb, z_tb)]):
                for it in range(half_t // XSC):
                    t0 = half * half_t + it * XSC
                    l0 = it * XSC  # tile within half
                    x_t = xsp.tile([P, XSC * D], f32, tag="x_t")
                    nc.sync.dma_start(
                        out=x_t[:].rearrange("p (c e) -> p c e", e=D),
                        in_=xown[:].rearrange("(c p) e -> p c e", p=P)[
                            :, t0:t0 + XSC, :])
                    xs_t = xsp.tile([P, XSC * D], bf16, tag="xs_t")
                    nc.vector.tensor_tensor(
                        out=xs_t[:].rearrange("p (c e) -> p c e", e=D),
                        in0=x_t[:].rearrange("p (c e) -> p c e", e=D),
                        in1=ns_f[:, t0:t0 + XSC, None].to_broadcast(
                            [P, XSC, D]),
                        op=OP.mult)
                    z_t = xsp.tile([P, XSC * D], bf16, tag="z_t")
                    for c in range(XSC):
                        tp_ps = psA.tile([P, P], bf16, tag="tp")
                        nc.tensor.transpose(
                            out=tp_ps[:], in_=xs_t[:, c * D:(c + 1) * D],
                            identity=ident[:])
                        xsT = trp.tile([P, P], bf16, tag="xsT")
                        nc.scalar.copy(xsT[:], tp_ps[:])
                        z_ps = psB.tile([P, P], f32, tag="B")
                        nc.tensor.matmul(out=z_ps[:], lhsT=xsT[:],
                                         rhs=w_sb[:], start=True, stop=True)
                        nc.vector.tensor_copy(z_t[:, c * D:(c + 1) * D],
                                              z_ps[:])
                    nc.scalar.dma_start(
                        out=cc_z[:].rearrange("(c p) e -> p c e", p=P)[
                            :, l0:l0 + XSC, :],
                        in_=z_t[:].rearrange("p (c e) -> p c e", e=D))
                nc.gpsimd.collective_compute(
                    "AllGather", OP.bypass,
                    replica_groups=[list(range(NCORES))],
                    ins=[cc_z[:]], outs=[z_t_out[:]])

            # ---- phase A ---------------------------------------------------
            h_store = storep.tile([P, nt * D], bf16, tag="hstore")
            sum_ps = psS.tile([1, P], f32, tag="sum")
            sq_ps = psS.tile([1, P], f32, tag="sq")

            def colbase(g, w):
                return g * sum(cols_gw) + sum(cols_gw[:w])

            def ebase(g, w):
                return g * GROUP * ktot + GROUP * sum(kws[:w])

            qn = 0
            for g in range(ngroups):
                e_ws = []
                s_ws = []
                for w in range(nwin):
                    kw = kws[w]
                    nidx = GROUP * kw * P
                    tab = z_ta if w < 2 else z_tb
                    wl = w % 2
                    e_t = gathp.tile([P, GROUP * kw * D], bf16, tag=f"E{w}")
                    nc.gpsimd.dma_gather(
                        e_t[:].rearrange("p (c e) -> p c e", e=D),
                        tab[wl * WIN:(wl + 1) * WIN, :],
                        idxs_sb[:, colbase(g, w):colbase(g, w) + nidx // 16],
                        nidx, nidx, D, single_packet=False,
                        queue_num=qn % 4)
                    qn += 1
                    e_ws.append(e_t)
                    nch = GROUP * kw
                    eb = ebase(g, w)
                    s_t = sp.tile([P, nch * P], bf16, tag=f"S{w}")
                    nc.vector.tensor_tensor(
                        out=s_t[:].rearrange("p (c e) -> p c e", e=P),
                        in0=iota[:, None, :].to_broadcast([P, nch, P]),
                        in1=eloc_sb[:, eb:eb + nch, None
                                    ].to_broadcast([P, nch, P]),
                        op=OP.is_equal)
                    s_ws.append(s_t)

                for u in range(GROUP):
                    t = g * GROUP + u
                    agg_ps = psA.tile([P, P], f32, tag="A")
                    ci = 0
                    for w in range(nwin):
                        kw = kws[w]
                        for j in range(kw):
                            nc.tensor.matmul(
                                out=agg_ps[:],
                                lhsT=s_ws[w][:, (u * kw + j) * P:
                                             (u * kw + j + 1) * P],
                                rhs=e_ws[w][:, (u * kw + j) * D:
                                            (u * kw + j + 1) * D],
                                start=(ci + j == 0),
                                stop=(ci + j == ktot - 1))
                        ci += kw
                    h_t = h_store[:, t * D:(t + 1) * D]
                    nc.vector.tensor_tensor(
                        out=h_t, in0=agg_ps[:],
                        in1=nd_f[:, t:t + 1].to_broadcast([P, D]),
                        op=OP.mult)
                    sq_sb = workp.tile([P, D], bf16, tag="sqsb")
                    nc.scalar.activation(out=sq_sb[:], in_=h_t,
                                         func=AF.Square)
                    nc.tensor.matmul(out=sum_ps[:], lhsT=ones_c[:], rhs=h_t,
                                     start=(t == 0), stop=(t == nt - 1))
                    nc.tensor.matmul(out=sq_ps[:], lhsT=ones_c[:],
                                     rhs=sq_sb[:],
                                     start=(t == 0), stop=(t == nt - 1))

            # ---- BN stats all-reduce + scale/shift (row layout) ------------
            srow = statp.tile([1, 2 * P], f32, tag="srow")
            nc.scalar.copy(srow[0:1, 0:P], sum_ps[:])
            nc.scalar.copy(srow[0:1, P:2 * P], sq_ps[:])
            cc_in = dramp.tile([1, 2 * P], f32, tag="ccin")
            cc_out = dramp.tile([1, 2 * P], f32, tag="ccout")
            nc.gpsimd.dma_start(out=cc_in[:], in_=srow[:])
            nc.gpsimd.collective_compute(
                "AllReduce", OP.add,
                replica_groups=[list(range(NCORES))],
                ins=[cc_in.opt()], outs=[cc_out.opt()])
            grow_sb = statp.tile([1, 2 * P], f32, tag="grow_sb")
            nc.gpsimd.dma_start(out=grow_sb[:], in_=cc_out[:])

            inv_n = 1.0 / float(n_real)
            mean_r = statp.tile([1, P], f32, tag="mean")
            nc.vector.tensor_scalar_mul(mean_r[:], grow_sb[0:1, 0:P], inv_n)
            ex2_r = statp.tile([1, P], f32, tag="ex2")
            nc.vector.tensor_scalar_mul(ex2_r[:], grow_sb[0:1, P:2 * P],
                                        inv_n)
            m2_r = statp.tile([1, P], f32, tag="m2")
            nc.scalar.square(m2_r[:], mean_r[:])
            var_r = statp.tile([1, P], f32, tag="var")
            nc.vector.tensor_tensor(out=var_r[:], in0=ex2_r[:], in1=m2_r[:],
                                    op=OP.subtract)
            nc.vector.tensor_scalar_add(var_r[:], var_r[:], BN_EPS)
            sd_r = statp.tile([1, P], f32, tag="sd")
            nc.scalar.sqrt(sd_r[:], var_r[:])
            inv_r = statp.tile([1, P], f32, tag="inv")
            nc.vector.reciprocal(inv_r[:], sd_r[:])
            sc_r = statp.tile([1, P], f32, tag="sc")
            nc.vector.tensor_tensor(out=sc_r[:], in0=g_row[:], in1=inv_r[:],
                                    op=OP.mult)
            # b cancels in BN: shift = beta - mean*scale
            tc_r = statp.tile([1, P], f32, tag="tc")
            nc.vector.tensor_tensor(out=tc_r[:], in0=mean_r[:], in1=sc_r[:],
                                    op=OP.mult)
            nc.vector.tensor_tensor(out=tc_r[:], in0=be_row[:], in1=tc_r[:],
                                    op=OP.subtract)

            # rank-1 broadcast of sc/tc across partitions -> bf16 tiles
            scb_ps = psA.tile([P, P], f32, tag="A")
            nc.tensor.matmul(out=scb_ps[:], lhsT=ones1[:], rhs=sc_r[:],
                             start=True, stop=True)
            sc_bc = constp.tile([P, P], bf16, tag="sc_bc")
            nc.scalar.copy(sc_bc[:], scb_ps[:])
            tcb_ps = psB.tile([P, P], f32, tag="B")
            nc.tensor.matmul(out=tcb_ps[:], lhsT=ones1[:], rhs=tc_r[:],
                             start=True, stop=True)
            tc_bc = constp.tile([P, P], bf16, tag="tc_bc")
            nc.scalar.copy(tc_bc[:], tcb_ps[:])

            # ---- phase B (batched loads/stores, no transposes) -------------
            for bt in range(nt // PBC):
                t0 = bt * PBC
                x_b = iop.tile([P, PBC * D], f32, tag="xb")
                nc.scalar.dma_start(
                    out=x_b[:].rearrange("p (c e) -> p c e", e=D),
                    in_=xown[:].rearrange("(c p) e -> p c e", p=P)[
                        :, t0:t0 + PBC, :])
                g1 = workp.tile([P, PBC * D], bf16, tag="g1")
                nc.vector.tensor_tensor(
                    out=g1[:].rearrange("p (c e) -> p c e", e=D),
                    in0=h_store[:, t0 * D:(t0 + PBC) * D].rearrange(
                        "p (c e) -> p c e", e=D),
                    in1=sc_bc[:, None, :].to_broadcast([P, PBC, D]),
                    op=OP.mult)
                g2 = workp.tile([P, PBC * D], bf16, tag="g2")
                nc.vector.tensor_tensor(
                    out=g2[:].rearrange("p (c e) -> p c e", e=D),
                    in0=g1[:].rearrange("p (c e) -> p c e", e=D),
                    in1=tc_bc[:, None, :].to_broadcast([P, PBC, D]),
                    op=OP.add)
                nc.vector.tensor_scalar_max(g2[:], g2[:], 0.0)
                nc.gpsimd.tensor_add(x_b[:], g2[:], x_b[:])
                nc.sync.dma_start(
                    out=out[:].rearrange("(c p) e -> p c e", p=P)[
                        :, t0:t0 + PBC, :],
                    in_=x_b[:].rearrange("p (c e) -> p c e", e=D))

    nc.compile()
    return nc


def _wrap16(a):
    b = a.reshape(-1, 16).T
    return np.tile(b, (8, 1))


def host_prep(x, src, dst, W, b, gamma, beta):
    """Graph routing / layout prep (indices only - no FLOPs on host)."""
    x = np.asarray(x, np.float32)
    W = np.asarray(W, np.float32)
    gamma = np.asarray(gamma, np.float32)
    beta = np.asarray(beta, np.float32)
    src32 = np.asarray(src).astype(np.int64)
    dst32 = np.asarray(dst).astype(np.int64)

    n = x.shape[0]
    npad = -(-n // (P * NCORES * XSC)) * (P * NCORES * XSC)
    nodes_pc = npad // NCORES
    nt = nodes_pc // P
    nt_tot = npad // P
    nwin = -(-npad // WIN)

    order = np.argsort(dst32, kind="stable")
    ds = dst32[order]
    ss = src32[order]

    ar = np.arange(npad + 1, dtype=np.int64)
    rps = np.searchsorted(np.sort(src32), ar).astype(np.int32)
    rpd_full = np.searchsorted(ds, ar)

    # split-AllGather table row for each src: half h of its core's slice,
    # table row = core*half_pc + (r - h*half_pc); window = 2h + row//WIN
    half_pc = nodes_pc // 2
    s_core = ss // nodes_pc
    s_r = ss % nodes_pc
    s_half = s_r // half_pc
    s_row = s_core * half_pc + (s_r - s_half * half_pc)
    s_winrow = s_row % WIN
    s_win = 2 * s_half + s_row // WIN

    # degree counts (int), F-order [P, nt] per core
    dgo_n = np.diff(rps).astype(np.int32)                 # [npad]
    dgi_n = np.diff(rpd_full).astype(np.int32)            # [npad]

    lo = rpd_full[np.arange(nt_tot) * P]
    hi = rpd_full[(np.arange(nt_tot) + 1) * P]

    # per (tile, window) edge lists
    sw = s_win
    kws = []
    tw_lists = [[None] * nwin for _ in range(nt_tot)]
    for w in range(nwin):
        cnt_max = 0
        for t in range(nt_tot):
            e = np.arange(lo[t], hi[t])
            sel = e[sw[lo[t]:hi[t]] == w]
            tw_lists[t][w] = sel
            cnt_max = max(cnt_max, len(sel))
        kws.append(-(-cnt_max // P))
    kws = tuple(kws)
    ktot = sum(kws)

    xpad = np.zeros((npad, D), np.float32)
    xpad[:n] = x

    import ml_dtypes
    ngroups = nt // GROUP
    in_maps = []
    shared = dict(
        wmat=W,
        grow=np.ascontiguousarray(gamma[None, :]),
        brow=np.ascontiguousarray(beta[None, :]))
    for c in range(NCORES):
        # eloc layout: (g, w, u, chunk) contiguous for batched S builds
        elocv = np.full((nt * ktot, P), -1.0, np.float32)
        idx_blocks = []
        ecol_off = 0
        for g in range(ngroups):
            for w in range(nwin):
                if kws[w] == 0:
                    continue
                blk = np.zeros(GROUP * kws[w] * P, np.int16)
                for u in range(GROUP):
                    t = g * GROUP + u
                    gt = c * nt + t
                    sel = tw_lists[gt][w]
                    base = u * kws[w] * P
                    blk[base:base + len(sel)] = s_winrow[sel].astype(np.int16)
                    ev = (ds[sel] - gt * P).astype(np.float32)
                    ecol = elocv[ecol_off + u * kws[w]:
                                 ecol_off + (u + 1) * kws[w]].reshape(-1)
                    ecol[:len(sel)] = ev
                ecol_off += GROUP * kws[w]
                idx_blocks.append(_wrap16(blk))
        m = dict(shared)
        m["xown"] = np.ascontiguousarray(xpad[c * nodes_pc:(c + 1) * nodes_pc])
        m["dgo"] = np.ascontiguousarray(
            dgo_n[c * nodes_pc:(c + 1) * nodes_pc].reshape(nt, P).T)
        m["dgi"] = np.ascontiguousarray(
            dgi_n[c * nodes_pc:(c + 1) * nodes_pc].reshape(nt, P).T)
        m["idxs"] = np.ascontiguousarray(np.concatenate(idx_blocks, axis=1))
        # eloc device layout: col (g,w,u,chunk) partition p = edge c*128+p
        m["eloc"] = np.ascontiguousarray(
            elocv.T).astype(ml_dtypes.bfloat16)
        in_maps.append(m)
    return dict(npad=npad, nt=nt, kws=kws, n_real=n), in_maps


def run(in_maps, cfg, **kw):
    from concourse.bass_utils import run_bass_kernel_spmd

    key = (cfg["npad"], cfg["nt"], tuple(cfg["kws"]), cfg["n_real"])
    if key not in _NC_CACHE:
        _NC_CACHE[key] = build_program(*key)
    nc = _NC_CACHE[key]
    res = run_bass_kernel_spmd(nc, in_maps, core_ids=list(range(NCORES)), **kw)
    n = cfg["n_real"]
    full = np.concatenate(
        [np.asarray(res.results[c]["out"]) for c in range(NCORES)],
        axis=0)[:n]
    return np.ascontiguousarray(full, dtype=np.float32), res


def kernel(x, src, dst, W, b, gamma, beta):
    cfg, in_maps = host_prep(x, src, dst, W, b, gamma, beta)
    out, _ = run(in_maps, cfg)
    return out



# revision 5
# speedup vs baseline: 1.0051x; 1.0051x over previous
"""DeepGCNLayer (GraphConv norm='both' + BatchNorm + ReLU + residual) on 8 trn2 cores.

Sharding: nodes padded to NPAD=100352, split into 8 ranges (98 node-tiles of
128 per core). Edges routed to the core owning their dst (dst-sorted), then
per (dst-tile, src-window) padded to a uniform K_w chunks of 128 so every
core runs one SPMD program.

v2 layout: the AllGather'd z table is built in 4 row-interleaved windows
(each window = the same quarter of every core's slice) so per-window
AllGathers overlap the z pass and the first gather groups. The one-hot S
matrices are built as ACT-engine broadcast expansion of eloc plus a DVE
is_equal on real tiles (2x perf mode, short shared-port holds) - the v1
broadcast tensor_tensor held the DVE shared SBUF port for ~28us/group,
starving the SWDGE gather descriptor generator (see trainium-docs
memories/01-sbuf.md "DVE blocks DMA" trap). x ships transposed bf16 so the
z pass needs no PE transposes; norm_src/norm_dst fold into ACT scale-copies.
"""

import sys

if "/opt/trn_rl_repo" not in sys.path:
    sys.path.insert(0, "/opt/trn_rl_repo")

import numpy as np

P = 128
D = 128
NCORES = 8
BN_EPS = 1e-5
GROUP = 7            # dst tiles per gather group
PBC = 7              # node tiles per phase-B load/store batch
WTILES = (25, 25, 24, 24)   # z-table window sizes in node tiles (per core)

_NC_CACHE = {}


def build_program(npad, nt, kws, n_real):
    """kws: tuple of chunks-per-window per dst tile (uniform across tiles)."""
    import concourse.bacc as bacc
    import concourse.tile as tile
    from concourse import mybir

    f32 = mybir.dt.float32
    bf16 = mybir.dt.bfloat16
    i32 = mybir.dt.int32
    i16 = mybir.dt.int16
    OP = mybir.AluOpType
    AF = mybir.ActivationFunctionType

    nodes_pc = nt * P
    ktot = sum(kws)
    nwin = len(kws)
    ngroups = nt // GROUP
    assert nt % GROUP == 0
    assert sum(WTILES) == nt and len(WTILES) == nwin
    kmax = max(kws)
    # idx16 columns per (group, window); eloc columns per (group, window)
    cols_gw = [GROUP * kw * P // 16 for kw in kws]
    gcols_i = sum(cols_gw)              # idx cols per group
    gcols_e = GROUP * ktot              # eloc cols per group

    nc = bacc.Bacc("TRN2", target_bir_lowering=False, debug=False,
                   num_devices=NCORES, num_swdge_queues=4)

    xt = nc.dram_tensor("xt", [P, nodes_pc], bf16, kind="ExternalInput")
    xres = nc.dram_tensor("xres", [nodes_pc, D], f32, kind="ExternalInput")
    wmat = nc.dram_tensor("wmat", [D, D], bf16, kind="ExternalInput")
    grow = nc.dram_tensor("grow", [1, D], f32, kind="ExternalInput")
    brow = nc.dram_tensor("brow", [1, D], f32, kind="ExternalInput")
    dgo = nc.dram_tensor("dgo", [P, nt], i32, kind="ExternalInput")
    dgi = nc.dram_tensor("dgi", [P, nt], i32, kind="ExternalInput")
    idxs = nc.dram_tensor("idxs", [P, ngroups * gcols_i], i16,
                          kind="ExternalInput")
    eloc = nc.dram_tensor("eloc", [P, ngroups * gcols_e], bf16,
                          kind="ExternalInput")
    out = nc.dram_tensor("out", [nodes_pc, D], f32, kind="ExternalOutput")

    # per-window z contribution + AllGather'd table (row-interleaved:
    # window w = [core0 quarter w | core1 quarter w | ...])
    cc_z = [nc.dram_tensor(f"cc_z{w}", [WTILES[w] * P, D], bf16)
            for w in range(nwin)]
    z_t = [nc.dram_tensor(f"z_t{w}", [WTILES[w] * P * NCORES, D], bf16,
                          addr_space="Shared")
           for w in range(nwin)]

    with tile.TileContext(nc) as tc:
        with (
            tc.tile_pool(name="const", bufs=1) as constp,
            tc.tile_pool(name="norm", bufs=1) as normp,
            tc.tile_pool(name="xz", bufs=1) as xzp,
            tc.tile_pool(name="zst", bufs=1) as zsp,
            tc.tile_pool(name="meta", bufs=2) as metap,
            tc.tile_pool(name="erep", bufs=2) as erepp,
            tc.tile_pool(name="gath", bufs=2) as gathp,
            tc.tile_pool(name="s", bufs=1) as sp,
            tc.tile_pool(name="work", bufs=2) as workp,
            tc.tile_pool(name="stats", bufs=1) as statp,
            tc.tile_pool(name="store", bufs=1) as storep,
            tc.tile_pool(name="io", bufs=2) as iop,
            tc.tile_pool(name="psA", bufs=2, space="PSUM") as psA,
            tc.tile_pool(name="psB", bufs=2, space="PSUM") as psB,
            tc.tile_pool(name="psS", bufs=1, space="PSUM") as psS,
            tc.tile_pool(name="dram", bufs=2, space="DRAM") as dramp,
        ):
            # ---- constants -------------------------------------------------
            iota = constp.tile([P, P], bf16, tag="iota")
            nc.gpsimd.iota(iota[:], pattern=[[1, P]], base=0,
                           channel_multiplier=0,
                           allow_small_or_imprecise_dtypes=True)
            # iota replicated along free dim for batched 2x is_equal
            iota_rep = constp.tile([P, GROUP * kmax * P], bf16, tag="iotar")
            nc.vector.tensor_copy(
                iota_rep[:].rearrange("p (c e) -> p c e", e=P),
                iota[:, None, :].to_broadcast([P, GROUP * kmax, P]))
            ones1 = constp.tile([1, P], f32, tag="ones1")
            nc.vector.memset(ones1[:], 1.0)
            ones_c = constp.tile([P, 1], bf16, tag="ones_c")
            nc.vector.memset(ones_c[:], 1.0)
            w_sb = constp.tile([P, D], bf16, tag="wsb")
            nc.sync.dma_start(out=w_sb[:], in_=wmat[:])
            g_row = constp.tile([1, D], f32, tag="grow")
            nc.sync.dma_start(out=g_row[:], in_=grow[:])
            be_row = constp.tile([1, D], f32, tag="berow")
            nc.sync.dma_start(out=be_row[:], in_=brow[:])

            # ---- norm arrays (own range, F-order [P, nt]) ------------------
            deg = normp.tile([P, nt], i32, tag="deg")
            nc.sync.dma_start(out=deg[:], in_=dgo[:])
            degf = normp.tile([P, nt], f32, tag="degf")
            nc.vector.tensor_scalar_max(degf[:], deg[:], 1.0)
            nc.scalar.sqrt(degf[:], degf[:])
            ns_f = constp.tile([P, nt], f32, tag="ns_f")
            nc.vector.reciprocal(ns_f[:], degf[:])

            deg2 = normp.tile([P, nt], i32, tag="deg2")
            nc.sync.dma_start(out=deg2[:], in_=dgi[:])
            deg2f = normp.tile([P, nt], f32, tag="deg2f")
            nc.vector.tensor_scalar_max(deg2f[:], deg2[:], 1.0)
            nc.scalar.sqrt(deg2f[:], deg2f[:])
            nd_f = constp.tile([P, nt], f32, tag="nd_f")
            nc.vector.reciprocal(nd_f[:], deg2f[:])

            # ---- z pass: z = (x@W)*ns -> bf16, own slice, 4 windows --------
            # lhsT = xT slice (no transposes); ns folds into the ACT
            # PSUM->SBUF copy; each window's AllGather issues as soon as its
            # quarter is stored so window-0 gathers start early.
            t0 = 0
            for w in range(nwin):
                wt = WTILES[w]
                xt_w = xzp.tile([P, wt * P], bf16, tag="xt_w")
                nc.sync.dma_start(out=xt_w[:],
                                  in_=xt[:, t0 * P:(t0 + wt) * P])
                z_w = zsp.tile([P, wt * D], bf16, tag="z_w")
                for c in range(wt):
                    z_ps = psB.tile([P, D], f32, tag="B")
                    nc.tensor.matmul(out=z_ps[:],
                                     lhsT=xt_w[:, c * P:(c + 1) * P],
                                     rhs=w_sb[:], start=True, stop=True)
                    nc.scalar.activation(
                        out=z_w[:, c * D:(c + 1) * D], in_=z_ps[:],
                        func=AF.Copy, scale=ns_f[:, t0 + c:t0 + c + 1])
                nc.scalar.dma_start(
                    out=cc_z[w][:].rearrange("(c p) e -> p c e", p=P),
                    in_=z_w[:].rearrange("p (c e) -> p c e", e=D))
                nc.gpsimd.collective_compute(
                    "AllGather", OP.bypass,
                    replica_groups=[list(range(NCORES))],
                    ins=[cc_z[w][:]], outs=[z_t[w][:]])
                t0 += wt

            # ---- phase A ---------------------------------------------------
            h_store = storep.tile([P, nt * D], bf16, tag="hstore")
            sum_ps = psS.tile([1, P], f32, tag="sum")
            sq_ps = psS.tile([1, P], f32, tag="sq")

            qn = 0
            for g in range(ngroups):
                idx_g = metap.tile([P, gcols_i], i16, tag="idxg")
                nc.sync.dma_start(
                    out=idx_g[:], in_=idxs[:, g * gcols_i:(g + 1) * gcols_i])
                eloc_g = metap.tile([P, gcols_e], bf16, tag="elocg")
                nc.sync.dma_start(
                    out=eloc_g[:], in_=eloc[:, g * gcols_e:(g + 1) * gcols_e])

                e_ws = []
                s_ws = []
                ico = 0
                eco = 0
                for w in range(nwin):
                    kw = kws[w]
                    nch = GROUP * kw
                    nidx = nch * P
                    e_t = gathp.tile([P, nch * D], bf16, tag=f"E{w}")
                    nc.gpsimd.dma_gather(
                        e_t[:].rearrange("p (c e) -> p c e", e=D),
                        z_t[w][:],
                        idx_g[:, ico:ico + nidx // 16],
                        nidx, nidx, D, single_packet=False,
                        queue_num=qn % 4)
                    qn += 1
                    ico += nidx // 16
                    e_ws.append(e_t)
                    # eloc_rep: ACT broadcast expansion (own SBUF port),
                    # then one batched 2x is_equal on real tiles (short
                    # shared-port hold).
                    el_r = erepp.tile([P, nch * P], bf16, tag="R")
                    nc.scalar.copy(
                        el_r[:].rearrange("p (c e) -> p c e", e=P),
                        eloc_g[:, eco:eco + nch, None].to_broadcast(
                            [P, nch, P]))
                    s_t = sp.tile([P, nch * P], bf16, tag=f"S{w}")
                    nc.vector.tensor_tensor(
                        out=s_t[:], in0=el_r[:],
                        in1=iota_rep[:, :nch * P], op=OP.is_equal)
                    eco += nch
                    s_ws.append(s_t)

                for u in range(GROUP):
                    t = g * GROUP + u
                    agg_ps = psA.tile([P, P], f32, tag="A")
                    ci = 0
                    for w in range(nwin):
                        kw = kws[w]
                        for j in range(kw):
                            nc.tensor.matmul(
                                out=agg_ps[:],
                                lhsT=s_ws[w][:, (u * kw + j) * P:
                                             (u * kw + j + 1) * P],
                                rhs=e_ws[w][:, (u * kw + j) * D:
                                            (u * kw + j + 1) * D],
                                start=(ci + j == 0),
                                stop=(ci + j == ktot - 1))
                        ci += kw
                    h_t = h_store[:, t * D:(t + 1) * D]
                    nc.scalar.activation(out=h_t, in_=agg_ps[:],
                                         func=AF.Copy,
                                         scale=nd_f[:, t:t + 1])
                    sq_sb = workp.tile([P, D], bf16, tag="sqsb")
                    nc.scalar.activation(out=sq_sb[:], in_=h_t,
                                         func=AF.Square)
                    nc.tensor.matmul(out=sum_ps[:], lhsT=ones_c[:], rhs=h_t,
                                     start=(t == 0), stop=(t == nt - 1))
                    nc.tensor.matmul(out=sq_ps[:], lhsT=ones_c[:],
                                     rhs=sq_sb[:],
                                     start=(t == 0), stop=(t == nt - 1))

            # ---- BN stats all-reduce + scale/shift (row layout) ------------
            srow = statp.tile([1, 2 * P], f32, tag="srow")
            nc.scalar.copy(srow[0:1, 0:P], sum_ps[:])
            nc.scalar.copy(srow[0:1, P:2 * P], sq_ps[:])
            cc_in = dramp.tile([1, 2 * P], f32, tag="ccin")
            cc_out = dramp.tile([1, 2 * P], f32, tag="ccout")
            nc.gpsimd.dma_start(out=cc_in[:], in_=srow[:])
            nc.gpsimd.collective_compute(
                "AllReduce", OP.add,
                replica_groups=[list(range(NCORES))],
                ins=[cc_in.opt()], outs=[cc_out.opt()])
            grow_sb = statp.tile([1, 2 * P], f32, tag="grow_sb")
            nc.gpsimd.dma_start(out=grow_sb[:], in_=cc_out[:])

            inv_n = 1.0 / float(n_real)
            mean_r = statp.tile([1, P], f32, tag="mean")
            nc.vector.tensor_scalar_mul(mean_r[:], grow_sb[0:1, 0:P], inv_n)
            ex2_r = statp.tile([1, P], f32, tag="ex2")
            nc.vector.tensor_scalar_mul(ex2_r[:], grow_sb[0:1, P:2 * P],
                                        inv_n)
            m2_r = statp.tile([1, P], f32, tag="m2")
            nc.scalar.square(m2_r[:], mean_r[:])
            var_r = statp.tile([1, P], f32, tag="var")
            nc.vector.tensor_tensor(out=var_r[:], in0=ex2_r[:], in1=m2_r[:],
                                    op=OP.subtract)
            nc.vector.tensor_scalar_add(var_r[:], var_r[:], BN_EPS)
            sd_r = statp.tile([1, P], f32, tag="sd")
            nc.scalar.sqrt(sd_r[:], var_r[:])
            inv_r = statp.tile([1, P], f32, tag="inv")
            nc.vector.reciprocal(inv_r[:], sd_r[:])
            sc_r = statp.tile([1, P], f32, tag="sc")
            nc.vector.tensor_tensor(out=sc_r[:], in0=g_row[:], in1=inv_r[:],
                                    op=OP.mult)
            # b cancels in BN: shift = beta - mean*scale
            tc_r = statp.tile([1, P], f32, tag="tc")
            nc.vector.tensor_tensor(out=tc_r[:], in0=mean_r[:], in1=sc_r[:],
                                    op=OP.mult)
            nc.vector.tensor_tensor(out=tc_r[:], in0=be_row[:], in1=tc_r[:],
                                    op=OP.subtract)

            # rank-1 broadcast of sc/tc across partitions -> bf16 tiles
            scb_ps = psA.tile([P, P], f32, tag="A")
            nc.tensor.matmul(out=scb_ps[:], lhsT=ones1[:], rhs=sc_r[:],
                             start=True, stop=True)
            sc_bc = constp.tile([P, P], bf16, tag="sc_bc")
            nc.scalar.copy(sc_bc[:], scb_ps[:])
            tcb_ps = psB.tile([P, P], f32, tag="B")
            nc.tensor.matmul(out=tcb_ps[:], lhsT=ones1[:], rhs=tc_r[:],
                             start=True, stop=True)
            tc_bc = constp.tile([P, P], bf16, tag="tc_bc")
            nc.scalar.copy(tc_bc[:], tcb_ps[:])

            # ---- phase B (batched loads/stores, no transposes) -------------
            for bt in range(nt // PBC):
                t0 = bt * PBC
                x_b = iop.tile([P, PBC * D], f32, tag="xb")
                nc.scalar.dma_start(
                    out=x_b[:].rearrange("p (c e) -> p c e", e=D),
                    in_=xres[:].rearrange("(c p) e -> p c e", p=P)[
                        :, t0:t0 + PBC, :])
                g1 = workp.tile([P, PBC * D], bf16, tag="g1")
                nc.vector.tensor_tensor(
                    out=g1[:].rearrange("p (c e) -> p c e", e=D),
                    in0=h_store[:, t0 * D:(t0 + PBC) * D].rearrange(
                        "p (c e) -> p c e", e=D),
                    in1=sc_bc[:, None, :].to_broadcast([P, PBC, D]),
                    op=OP.mult)
                nc.vector.tensor_tensor(
                    out=g1[:].rearrange("p (c e) -> p c e", e=D),
                    in0=g1[:].rearrange("p (c e) -> p c e", e=D),
                    in1=tc_bc[:, None, :].to_broadcast([P, PBC, D]),
                    op=OP.add)
                nc.vector.tensor_scalar_max(g1[:], g1[:], 0.0)
                nc.vector.tensor_tensor(out=x_b[:], in0=g1[:], in1=x_b[:],
                                        op=OP.add)
                nc.sync.dma_start(
                    out=out[:].rearrange("(c p) e -> p c e", p=P)[
                        :, t0:t0 + PBC, :],
                    in_=x_b[:].rearrange("p (c e) -> p c e", e=D))

    nc.compile()
    return nc


def _wrap16(a):
    b = a.reshape(-1, 16).T
    return np.tile(b, (8, 1))


def host_prep(x, src, dst, W, b, gamma, beta):
    """Graph routing / layout prep (indices only - no FLOPs on host)."""
    import ml_dtypes

    x = np.asarray(x, np.float32)
    W = np.asarray(W, np.float32)
    gamma = np.asarray(gamma, np.float32)
    beta = np.asarray(beta, np.float32)
    src32 = np.asarray(src).astype(np.int64)
    dst32 = np.asarray(dst).astype(np.int64)

    n = x.shape[0]
    npad = -(-n // (P * NCORES * GROUP)) * (P * NCORES * GROUP)
    nodes_pc = npad // NCORES
    nt = nodes_pc // P
    nt_tot = npad // P
    assert sum(WTILES) == nt
    nwin = len(WTILES)
    wt_start = np.cumsum([0] + list(WTILES))  # in tiles, per core

    order = np.argsort(dst32, kind="stable")
    ds = dst32[order]
    ss = src32[order]

    ar = np.arange(npad + 1, dtype=np.int64)
    rps = np.searchsorted(np.sort(src32), ar).astype(np.int32)
    rpd_full = np.searchsorted(ds, ar)

    # src -> (window, row within window table). Window w of the z table is
    # [core0 quarter w | core1 quarter w | ...], quarter w = tiles
    # [wt_start[w], wt_start[w+1]) of each core's slice.
    s_core = ss // nodes_pc
    s_r = ss % nodes_pc
    s_tile = s_r // P
    s_win = np.searchsorted(wt_start, s_tile, side="right") - 1
    wrows = (np.array(WTILES) * P)[s_win]
    s_winrow = s_core * wrows + (s_r - wt_start[s_win] * P)

    # degree counts (int), F-order [P, nt] per core
    dgo_n = np.diff(rps).astype(np.int32)                 # [npad]
    dgi_n = np.diff(rpd_full).astype(np.int32)            # [npad]

    lo = rpd_full[np.arange(nt_tot) * P]
    hi = rpd_full[(np.arange(nt_tot) + 1) * P]

    # per (tile, window) edge lists
    kws = []
    tw_lists = [[None] * nwin for _ in range(nt_tot)]
    for w in range(nwin):
        cnt_max = 0
        for t in range(nt_tot):
            e = np.arange(lo[t], hi[t])
            sel = e[s_win[lo[t]:hi[t]] == w]
            tw_lists[t][w] = sel
            cnt_max = max(cnt_max, len(sel))
        kws.append(-(-cnt_max // P))
    kws = tuple(kws)
    ktot = sum(kws)

    xpad = np.zeros((npad, D), np.float32)
    xpad[:n] = x

    ngroups = nt // GROUP
    in_maps = []
    shared = dict(
        wmat=W.astype(ml_dtypes.bfloat16),
        grow=np.ascontiguousarray(gamma[None, :]),
        brow=np.ascontiguousarray(beta[None, :]))
    for c in range(NCORES):
        # eloc layout: (g, w, u, chunk) contiguous for batched expansions
        elocv = np.full((nt * ktot, P), -1.0, np.float32)
        idx_blocks = []
        ecol_off = 0
        for g in range(ngroups):
            for w in range(nwin):
                blk = np.zeros(GROUP * kws[w] * P, np.int16)
                for u in range(GROUP):
                    t = g * GROUP + u
                    gt = c * nt + t
                    sel = tw_lists[gt][w]
                    base = u * kws[w] * P
                    blk[base:base + len(sel)] = s_winrow[sel].astype(np.int16)
                    ev = (ds[sel] - gt * P).astype(np.float32)
                    ecol = elocv[ecol_off + u * kws[w]:
                                 ecol_off + (u + 1) * kws[w]].reshape(-1)
                    ecol[:len(sel)] = ev
                ecol_off += GROUP * kws[w]
                idx_blocks.append(_wrap16(blk))
        m = dict(shared)
        xslice = xpad[c * nodes_pc:(c + 1) * nodes_pc]
        m["xt"] = np.ascontiguousarray(xslice.T).astype(ml_dtypes.bfloat16)
        m["xres"] = np.ascontiguousarray(xslice)
        m["dgo"] = np.ascontiguousarray(
            dgo_n[c * nodes_pc:(c + 1) * nodes_pc].reshape(nt, P).T)
        m["dgi"] = np.ascontiguousarray(
            dgi_n[c * nodes_pc:(c + 1) * nodes_pc].reshape(nt, P).T)
        m["idxs"] = np.ascontiguousarray(np.concatenate(idx_blocks, axis=1))
        # eloc device layout: col (g,w,u,chunk) partition p = edge c*128+p
        m["eloc"] = np.ascontiguousarray(
            elocv.T).astype(ml_dtypes.bfloat16)
        in_maps.append(m)
    return dict(npad=npad, nt=nt, kws=kws, n_real=n), in_maps


def run(in_maps, cfg, **kw):
    from concourse.bass_utils import run_bass_kernel_spmd

    key = (cfg["npad"], cfg["nt"], tuple(cfg["kws"]), cfg["n_real"])
    if key not in _NC_CACHE:
        _NC_CACHE[key] = build_program(*key)
    nc = _NC_CACHE[key]
    res = run_bass_kernel_spmd(nc, in_maps, core_ids=list(range(NCORES)), **kw)
    n = cfg["n_real"]
    full = np.concatenate(
        [np.asarray(res.results[c]["out"]) for c in range(NCORES)],
        axis=0)[:n]
    return np.ascontiguousarray(full, dtype=np.float32), res


def kernel(x, src, dst, W, b, gamma, beta):
    cfg, in_maps = host_prep(x, src, dst, W, b, gamma, beta)
    out, _ = run(in_maps, cfg)
    return out


# revision 18
# speedup vs baseline: 1.0742x; 1.0688x over previous
"""DeepGCNLayer (GraphConv norm='both' + BatchNorm + ReLU + residual) on 8 trn2 cores.

Sharding: nodes padded to NPAD=100352, split into 8 ranges (98 node-tiles of
128 per core). Edges routed to the core owning their dst (dst-sorted), then
per (dst-tile, src-window) padded to a uniform K_w chunks of 128 so every
core runs one SPMD program.

v2 layout: the AllGather'd z table is built in 4 row-interleaved windows
(each window = the same quarter of every core's slice) so per-window
AllGathers overlap the z pass and the first gather groups. The one-hot S
matrices are built as ACT-engine broadcast expansion of eloc plus a DVE
is_equal on real tiles (2x perf mode, short shared-port holds) - the v1
broadcast tensor_tensor held the DVE shared SBUF port for ~28us/group,
starving the SWDGE gather descriptor generator (see trainium-docs
memories/01-sbuf.md "DVE blocks DMA" trap). x ships transposed bf16 so the
z pass needs no PE transposes; norm_src/norm_dst fold into ACT scale-copies.
"""

import sys

if "/opt/trn_rl_repo" not in sys.path:
    sys.path.insert(0, "/opt/trn_rl_repo")

import numpy as np

P = 128
D = 128
NCORES = 8
BN_EPS = 1e-5
GROUP = 7            # dst tiles per gather group
PBC = 7              # node tiles per phase-B load/store batch
WTILES = (27, 27, 22, 22)   # z-table window sizes in node tiles (per core)

_NC_CACHE = {}


def build_program(npad, nt, kws, n_real):
    """kws: tuple of chunks-per-window per dst tile (uniform across tiles)."""
    import concourse.bacc as bacc
    import concourse.tile as tile
    from concourse import mybir

    f32 = mybir.dt.float32
    bf16 = mybir.dt.bfloat16
    i32 = mybir.dt.int32
    i16 = mybir.dt.int16
    OP = mybir.AluOpType
    AF = mybir.ActivationFunctionType

    nodes_pc = nt * P
    ktot = sum(kws)
    nwin = len(kws)
    ngroups = nt // GROUP
    assert nt % GROUP == 0
    assert sum(WTILES) == nt and len(WTILES) == nwin
    kmax = max(kws)
    # idx16 columns per (group, window); eloc columns per (group, window)
    cols_gw = [GROUP * kw * P // 16 for kw in kws]
    gcols_i = sum(cols_gw)              # idx cols per group
    gcols_e = GROUP * ktot              # eloc cols per group

    nc = bacc.Bacc("TRN2", target_bir_lowering=False, debug=False,
                   num_devices=NCORES, num_swdge_queues=4)

    xt = nc.dram_tensor("xt", [P, nodes_pc], bf16, kind="ExternalInput")
    xres = nc.dram_tensor("xres", [nodes_pc, D], f32, kind="ExternalInput")
    wmat = nc.dram_tensor("wmat", [D, D], bf16, kind="ExternalInput")
    grow = nc.dram_tensor("grow", [1, D], f32, kind="ExternalInput")
    brow = nc.dram_tensor("brow", [1, D], f32, kind="ExternalInput")
    dgo = nc.dram_tensor("dgo", [P, nt], i32, kind="ExternalInput")
    dgi = nc.dram_tensor("dgi", [P, nt], i32, kind="ExternalInput")
    idxs = nc.dram_tensor("idxs", [P, ngroups * gcols_i], i16,
                          kind="ExternalInput")
    eloc = nc.dram_tensor("eloc", [P, ngroups * gcols_e], bf16,
                          kind="ExternalInput")
    out = nc.dram_tensor("out", [nodes_pc, D], f32, kind="ExternalOutput")
    h_d = nc.dram_tensor("h_d", [nodes_pc, D], bf16)

    # per-window z contribution + AllGather'd table (row-interleaved:
    # window w = [core0 quarter w | core1 quarter w | ...])
    cc_z = [nc.dram_tensor(f"cc_z{w}", [WTILES[w] * P, D], bf16)
            for w in range(nwin)]
    z_t = [nc.dram_tensor(f"z_t{w}", [WTILES[w] * P * NCORES, D], bf16,
                          addr_space="Shared")
           for w in range(nwin)]

    with tile.TileContext(nc) as tc:
        with (
            tc.tile_pool(name="const", bufs=1) as constp,
            tc.tile_pool(name="norm", bufs=1) as normp,
            tc.tile_pool(name="xz", bufs=1) as xzp,
            tc.tile_pool(name="zst", bufs=1) as zsp,
            tc.tile_pool(name="meta", bufs=2) as metap,
            tc.tile_pool(name="erep", bufs=1) as erepp,
            tc.tile_pool(name="gathA", bufs=3) as gathA,
            tc.tile_pool(name="gathB", bufs=2) as gathB,
            tc.tile_pool(name="s", bufs=1) as sp,
            tc.tile_pool(name="work", bufs=2) as workp,
            tc.tile_pool(name="stats", bufs=1) as statp,
            tc.tile_pool(name="io", bufs=3) as iop,
            tc.tile_pool(name="psA", bufs=2, space="PSUM") as psA,
            tc.tile_pool(name="psB", bufs=2, space="PSUM") as psB,
            tc.tile_pool(name="psS", bufs=1, space="PSUM") as psS,
            tc.tile_pool(name="dram", bufs=2, space="DRAM") as dramp,
        ):
            # ---- constants -------------------------------------------------
            iota = constp.tile([P, P], bf16, tag="iota")
            nc.gpsimd.iota(iota[:], pattern=[[1, P]], base=0,
                           channel_multiplier=0,
                           allow_small_or_imprecise_dtypes=True)
            # iota replicated along free dim for batched 2x is_equal
            iota_rep = constp.tile([P, GROUP * kmax * P], bf16, tag="iotar")
            nc.vector.tensor_copy(
                iota_rep[:].rearrange("p (c e) -> p c e", e=P),
                iota[:, None, :].to_broadcast([P, GROUP * kmax, P]))
            ones1 = constp.tile([1, P], f32, tag="ones1")
            nc.vector.memset(ones1[:], 1.0)
            ones_c = constp.tile([P, 1], bf16, tag="ones_c")
            nc.vector.memset(ones_c[:], 1.0)
            w_sb = constp.tile([P, D], bf16, tag="wsb")
            nc.sync.dma_start(out=w_sb[:], in_=wmat[:])
            g_row = constp.tile([1, D], f32, tag="grow")
            nc.sync.dma_start(out=g_row[:], in_=grow[:])
            be_row = constp.tile([1, D], f32, tag="berow")
            nc.sync.dma_start(out=be_row[:], in_=brow[:])

            # ---- norm arrays (own range, F-order [P, nt]) ------------------
            deg = normp.tile([P, nt], i32, tag="deg")
            nc.sync.dma_start(out=deg[:], in_=dgo[:])
            degf = normp.tile([P, nt], f32, tag="degf")
            nc.vector.tensor_scalar_max(degf[:], deg[:], 1.0)
            nc.scalar.sqrt(degf[:], degf[:])
            ns_f = constp.tile([P, nt], f32, tag="ns_f")
            nc.vector.reciprocal(ns_f[:], degf[:])

            deg2 = normp.tile([P, nt], i32, tag="deg2")
            nc.sync.dma_start(out=deg2[:], in_=dgi[:])
            deg2f = normp.tile([P, nt], f32, tag="deg2f")
            nc.vector.tensor_scalar_max(deg2f[:], deg2[:], 1.0)
            nc.scalar.sqrt(deg2f[:], deg2f[:])
            nd_f = constp.tile([P, nt], f32, tag="nd_f")
            nc.vector.reciprocal(nd_f[:], deg2f[:])

            # ---- z pass: z = (x@W)*ns -> bf16, own slice, 4 windows --------
            # lhsT = xT slice (no transposes); ns folds into the ACT
            # PSUM->SBUF copy; each window's AllGather issues as soon as its
            # quarter is stored so window-0 gathers start early.
            t0 = 0
            for w in range(nwin):
                wt = WTILES[w]
                xt_w = xzp.tile([P, wt * P], bf16, tag="xt_w")
                nc.sync.dma_start(out=xt_w[:],
                                  in_=xt[:, t0 * P:(t0 + wt) * P])
                z_w = zsp.tile([P, wt * D], bf16, tag="z_w")
                for c in range(wt):
                    z_ps = psB.tile([P, D], f32, tag="B")
                    nc.tensor.matmul(out=z_ps[:],
                                     lhsT=xt_w[:, c * P:(c + 1) * P],
                                     rhs=w_sb[:], start=True, stop=True)
                    nc.scalar.activation(
                        out=z_w[:, c * D:(c + 1) * D], in_=z_ps[:],
                        func=AF.Copy, scale=ns_f[:, t0 + c:t0 + c + 1])
                nc.scalar.dma_start(
                    out=cc_z[w][:].rearrange("(c p) e -> p c e", p=P),
                    in_=z_w[:].rearrange("p (c e) -> p c e", e=D))
                nc.gpsimd.collective_compute(
                    "AllGather", OP.bypass,
                    replica_groups=[list(range(NCORES))],
                    ins=[cc_z[w][:]], outs=[z_t[w][:]])
                t0 += wt

            # ---- phase A ---------------------------------------------------
            sum_ps = psS.tile([1, P], f32, tag="sum")
            sq_ps = psS.tile([1, P], f32, tag="sq")

            for g in range(ngroups):
                idx_g = metap.tile([P, gcols_i], i16, tag="idxg")
                nc.sync.dma_start(
                    out=idx_g[:], in_=idxs[:, g * gcols_i:(g + 1) * gcols_i])
                eloc_g = metap.tile([P, gcols_e], bf16, tag="elocg")
                nc.sync.dma_start(
                    out=eloc_g[:], in_=eloc[:, g * gcols_e:(g + 1) * gcols_e])

                e_ws = []
                s_ws = []
                ico = 0
                eco = 0
                for w in range(nwin):
                    kw = kws[w]
                    nch = GROUP * kw
                    nidx = nch * P
                    pool = gathA if w < 2 else gathB
                    e_t = pool.tile([P, nch * D], bf16, tag=f"E{w}")
                    nc.gpsimd.dma_gather(
                        e_t[:].rearrange("p (c e) -> p c e", e=D),
                        z_t[w][:],
                        idx_g[:, ico:ico + nidx // 16],
                        nidx, nidx, D, single_packet=False,
                        queue_num=(g + w) % 4)
                    ico += nidx // 16
                    e_ws.append(e_t)
                    # eloc_rep: ACT broadcast expansion (own SBUF port),
                    # then one batched 2x is_equal on real tiles (short
                    # shared-port hold).
                    el_r = erepp.tile([P, nch * P], bf16, tag="R")
                    nc.scalar.copy(
                        el_r[:].rearrange("p (c e) -> p c e", e=P),
                        eloc_g[:, eco:eco + nch, None].to_broadcast(
                            [P, nch, P]))
                    s_t = sp.tile([P, nch * P], bf16, tag=f"S{w}")
                    nc.vector.tensor_tensor(
                        out=s_t[:], in0=el_r[:],
                        in1=iota_rep[:, :nch * P], op=OP.is_equal)
                    eco += nch
                    s_ws.append(s_t)

                h_g = workp.tile([P, GROUP * D], bf16, tag="hg")
                for u in range(GROUP):
                    t = g * GROUP + u
                    agg_ps = psA.tile([P, P], f32, tag="A")
                    ci = 0
                    for w in range(nwin):
                        kw = kws[w]
                        for j in range(kw):
                            nc.tensor.matmul(
                                out=agg_ps[:],
                                lhsT=s_ws[w][:, (u * kw + j) * P:
                                             (u * kw + j + 1) * P],
                                rhs=e_ws[w][:, (u * kw + j) * D:
                                            (u * kw + j + 1) * D],
                                start=(ci + j == 0),
                                stop=(ci + j == ktot - 1))
                        ci += kw
                    h_t = h_g[:, u * D:(u + 1) * D]
                    nc.scalar.activation(out=h_t, in_=agg_ps[:],
                                         func=AF.Copy,
                                         scale=nd_f[:, t:t + 1])
                    sq_sb = workp.tile([P, D], bf16, tag="sqsb")
                    nc.scalar.activation(out=sq_sb[:], in_=h_t,
                                         func=AF.Square)
                    nc.tensor.matmul(out=sum_ps[:], lhsT=ones_c[:], rhs=h_t,
                                     start=(t == 0), stop=(t == nt - 1))
                    nc.tensor.matmul(out=sq_ps[:], lhsT=ones_c[:],
                                     rhs=sq_sb[:],
                                     start=(t == 0), stop=(t == nt - 1))
                nc.scalar.dma_start(
                    out=h_d[:].rearrange("(c p) e -> p c e", p=P)[
                        :, g * GROUP:(g + 1) * GROUP, :],
                    in_=h_g[:].rearrange("p (c e) -> p c e", e=D))

            # ---- BN stats all-reduce + scale/shift (row layout) ------------
            srow = statp.tile([1, 2 * P], f32, tag="srow")
            nc.scalar.copy(srow[0:1, 0:P], sum_ps[:])
            nc.scalar.copy(srow[0:1, P:2 * P], sq_ps[:])
            cc_in = dramp.tile([1, 2 * P], f32, tag="ccin")
            cc_out = dramp.tile([1, 2 * P], f32, tag="ccout")
            nc.gpsimd.dma_start(out=cc_in[:], in_=srow[:])
            nc.gpsimd.collective_compute(
                "AllReduce", OP.add,
                replica_groups=[list(range(NCORES))],
                ins=[cc_in.opt()], outs=[cc_out.opt()])
            grow_sb = statp.tile([1, 2 * P], f32, tag="grow_sb")
            nc.gpsimd.dma_start(out=grow_sb[:], in_=cc_out[:])

            inv_n = 1.0 / float(n_real)
            mean_r = statp.tile([1, P], f32, tag="mean")
            nc.vector.tensor_scalar_mul(mean_r[:], grow_sb[0:1, 0:P], inv_n)
            ex2_r = statp.tile([1, P], f32, tag="ex2")
            nc.vector.tensor_scalar_mul(ex2_r[:], grow_sb[0:1, P:2 * P],
                                        inv_n)
            m2_r = statp.tile([1, P], f32, tag="m2")
            nc.scalar.square(m2_r[:], mean_r[:])
            var_r = statp.tile([1, P], f32, tag="var")
            nc.vector.tensor_tensor(out=var_r[:], in0=ex2_r[:], in1=m2_r[:],
                                    op=OP.subtract)
            nc.vector.tensor_scalar_add(var_r[:], var_r[:], BN_EPS)
            sd_r = statp.tile([1, P], f32, tag="sd")
            nc.scalar.sqrt(sd_r[:], var_r[:])
            inv_r = statp.tile([1, P], f32, tag="inv")
            nc.vector.reciprocal(inv_r[:], sd_r[:])
            sc_r = statp.tile([1, P], f32, tag="sc")
            nc.vector.tensor_tensor(out=sc_r[:], in0=g_row[:], in1=inv_r[:],
                                    op=OP.mult)
            # b cancels in BN: shift = beta - mean*scale
            tc_r = statp.tile([1, P], f32, tag="tc")
            nc.vector.tensor_tensor(out=tc_r[:], in0=mean_r[:], in1=sc_r[:],
                                    op=OP.mult)
            nc.vector.tensor_tensor(out=tc_r[:], in0=be_row[:], in1=tc_r[:],
                                    op=OP.subtract)

            # rank-1 broadcast of sc/tc across partitions -> bf16 tiles
            scb_ps = psA.tile([P, P], f32, tag="A")
            nc.tensor.matmul(out=scb_ps[:], lhsT=ones1[:], rhs=sc_r[:],
                             start=True, stop=True)
            sc_bc = constp.tile([P, P], bf16, tag="sc_bc")
            nc.scalar.copy(sc_bc[:], scb_ps[:])
            tcb_ps = psB.tile([P, P], f32, tag="B")
            nc.tensor.matmul(out=tcb_ps[:], lhsT=ones1[:], rhs=tc_r[:],
                             start=True, stop=True)
            tc_bc = constp.tile([P, P], bf16, tag="tc_bc")
            nc.scalar.copy(tc_bc[:], tcb_ps[:])

            # ---- phase B (batched loads/stores, no transposes) -------------
            for bt in range(nt // PBC):
                t0 = bt * PBC
                x_b = iop.tile([P, PBC * D], f32, tag="xb")
                nc.scalar.dma_start(
                    out=x_b[:].rearrange("p (c e) -> p c e", e=D),
                    in_=xres[:].rearrange("(c p) e -> p c e", p=P)[
                        :, t0:t0 + PBC, :])
                h_b = iop.tile([P, PBC * D], bf16, tag="hb")
                nc.sync.dma_start(
                    out=h_b[:].rearrange("p (c e) -> p c e", e=D),
                    in_=h_d[:].rearrange("(c p) e -> p c e", p=P)[
                        :, t0:t0 + PBC, :])
                g1 = workp.tile([P, PBC * D], bf16, tag="g1")
                nc.vector.tensor_tensor(
                    out=g1[:].rearrange("p (c e) -> p c e", e=D),
                    in0=h_b[:].rearrange("p (c e) -> p c e", e=D),
                    in1=sc_bc[:, None, :].to_broadcast([P, PBC, D]),
                    op=OP.mult)
                nc.vector.tensor_tensor(
                    out=g1[:].rearrange("p (c e) -> p c e", e=D),
                    in0=g1[:].rearrange("p (c e) -> p c e", e=D),
                    in1=tc_bc[:, None, :].to_broadcast([P, PBC, D]),
                    op=OP.add)
                nc.vector.tensor_scalar_max(g1[:], g1[:], 0.0)
                nc.vector.tensor_tensor(out=x_b[:], in0=g1[:], in1=x_b[:],
                                        op=OP.add)
                nc.sync.dma_start(
                    out=out[:].rearrange("(c p) e -> p c e", p=P)[
                        :, t0:t0 + PBC, :],
                    in_=x_b[:].rearrange("p (c e) -> p c e", e=D))

    nc.compile()
    return nc


def _wrap16(a):
    b = a.reshape(-1, 16).T
    return np.tile(b, (8, 1))


def host_prep(x, src, dst, W, b, gamma, beta):
    """Graph routing / layout prep (indices only - no FLOPs on host)."""
    import ml_dtypes

    x = np.asarray(x, np.float32)
    W = np.asarray(W, np.float32)
    gamma = np.asarray(gamma, np.float32)
    beta = np.asarray(beta, np.float32)
    src32 = np.asarray(src).astype(np.int64)
    dst32 = np.asarray(dst).astype(np.int64)

    n = x.shape[0]
    npad = -(-n // (P * NCORES * GROUP)) * (P * NCORES * GROUP)
    nodes_pc = npad // NCORES
    nt = nodes_pc // P
    nt_tot = npad // P
    assert sum(WTILES) == nt
    nwin = len(WTILES)
    wt_start = np.cumsum([0] + list(WTILES))  # in tiles, per core

    order = np.argsort(dst32, kind="stable")
    ds = dst32[order]
    ss = src32[order]

    ar = np.arange(npad + 1, dtype=np.int64)
    rps = np.searchsorted(np.sort(src32), ar).astype(np.int32)
    rpd_full = np.searchsorted(ds, ar)

    # src -> (window, row within window table). Window w of the z table is
    # [core0 quarter w | core1 quarter w | ...], quarter w = tiles
    # [wt_start[w], wt_start[w+1]) of each core's slice.
    s_core = ss // nodes_pc
    s_r = ss % nodes_pc
    s_tile = s_r // P
    s_win = np.searchsorted(wt_start, s_tile, side="right") - 1
    wrows = (np.array(WTILES) * P)[s_win]
    s_winrow = s_core * wrows + (s_r - wt_start[s_win] * P)

    # degree counts (int), F-order [P, nt] per core
    dgo_n = np.diff(rps).astype(np.int32)                 # [npad]
    dgi_n = np.diff(rpd_full).astype(np.int32)            # [npad]

    # per-dst in-degree split by src window
    deg4 = np.zeros((npad, nwin), np.int32)
    np.add.at(deg4, (ds, s_win), 1)

    # --- bin-pack dst nodes into tiles (per core) to flatten the
    # per-(tile, window) edge-count tails, so kws (chunk counts) shrink.
    caps = np.array([5 * P, 5 * P, 4 * P, 4 * P], np.float64)
    newpos = np.empty(npad, np.int64)    # global node -> permuted local slot
    for c in range(NCORES):
        d4 = deg4[c * nodes_pc:(c + 1) * nodes_pc].astype(np.float64)
        order_c = np.argsort(-d4.sum(1), kind="stable")
        loads = np.zeros((nt, nwin))
        counts = np.zeros(nt, np.int64)
        fill = [[] for _ in range(nt)]
        for i in order_c:
            util = np.max((loads + d4[i]) / caps, axis=1)
            util[counts >= P] = np.inf
            b = int(np.argmin(util))
            loads[b] += d4[i]
            counts[b] += 1
            fill[b].append(i)
        pos = np.empty(nodes_pc, np.int64)
        for b in range(nt):
            pos[np.array(fill[b], np.int64)] = (
                b * P + np.arange(len(fill[b])))
        newpos[c * nodes_pc:(c + 1) * nodes_pc] = pos

    # kws from the packed loads (global max over cores/tiles per window)
    e_core = ds // nodes_pc
    e_pos = newpos[ds]
    e_tile = e_core * nt + e_pos // P
    e_slot = e_pos % P
    cell = e_tile * nwin + s_win
    cnt = np.bincount(cell, minlength=nt_tot * nwin).reshape(nt_tot, nwin)
    kws = tuple(int(-(-cnt[:, w].max() // P)) for w in range(nwin))
    ktot = sum(kws)

    # per (tile, window) edge lists under the permutation
    eorder = np.argsort(cell, kind="stable")
    bnd = np.searchsorted(cell[eorder], np.arange(nt_tot * nwin + 1))
    tw_lists = [[eorder[bnd[t * nwin + w]:bnd[t * nwin + w + 1]]
                 for w in range(nwin)] for t in range(nt_tot)]

    xpad = np.zeros((npad, D), np.float32)
    xpad[:n] = x

    ngroups = nt // GROUP
    in_maps = []
    shared = dict(
        wmat=W.astype(ml_dtypes.bfloat16),
        grow=np.ascontiguousarray(gamma[None, :]),
        brow=np.ascontiguousarray(beta[None, :]))
    for c in range(NCORES):
        # eloc layout: (g, w, u, chunk) contiguous for batched expansions
        elocv = np.full((nt * ktot, P), -1.0, np.float32)
        idx_blocks = []
        ecol_off = 0
        for g in range(ngroups):
            for w in range(nwin):
                blk = np.zeros(GROUP * kws[w] * P, np.int16)
                for u in range(GROUP):
                    t = g * GROUP + u
                    gt = c * nt + t
                    sel = tw_lists[gt][w]
                    base = u * kws[w] * P
                    blk[base:base + len(sel)] = s_winrow[sel].astype(np.int16)
                    ev = e_slot[sel].astype(np.float32)
                    ecol = elocv[ecol_off + u * kws[w]:
                                 ecol_off + (u + 1) * kws[w]].reshape(-1)
                    ecol[:len(sel)] = ev
                ecol_off += GROUP * kws[w]
                idx_blocks.append(_wrap16(blk))
        m = dict(shared)
        xslice = xpad[c * nodes_pc:(c + 1) * nodes_pc]
        pos_c = newpos[c * nodes_pc:(c + 1) * nodes_pc]
        orig_of = np.empty(nodes_pc, np.int64)
        orig_of[pos_c] = np.arange(nodes_pc)
        m["xt"] = np.ascontiguousarray(xslice.T).astype(ml_dtypes.bfloat16)
        m["xres"] = np.ascontiguousarray(xslice[orig_of])
        m["dgo"] = np.ascontiguousarray(
            dgo_n[c * nodes_pc:(c + 1) * nodes_pc].reshape(nt, P).T)
        m["dgi"] = np.ascontiguousarray(
            dgi_n[c * nodes_pc:(c + 1) * nodes_pc][orig_of]
            .reshape(nt, P).T)
        m["idxs"] = np.ascontiguousarray(np.concatenate(idx_blocks, axis=1))
        # eloc device layout: col (g,w,u,chunk) partition p = edge c*128+p
        m["eloc"] = np.ascontiguousarray(
            elocv.T).astype(ml_dtypes.bfloat16)
        in_maps.append(m)
    return dict(npad=npad, nt=nt, kws=kws, n_real=n, newpos=newpos), in_maps


def run(in_maps, cfg, **kw):
    from concourse.bass_utils import run_bass_kernel_spmd

    key = (cfg["npad"], cfg["nt"], tuple(cfg["kws"]), cfg["n_real"])
    if key not in _NC_CACHE:
        _NC_CACHE[key] = build_program(*key)
    nc = _NC_CACHE[key]
    res = run_bass_kernel_spmd(nc, in_maps, core_ids=list(range(NCORES)), **kw)
    n = cfg["n_real"]
    nodes_pc = cfg["npad"] // NCORES
    parts = []
    for c in range(NCORES):
        o = np.asarray(res.results[c]["out"])
        pos_c = cfg["newpos"][c * nodes_pc:(c + 1) * nodes_pc]
        parts.append(o[pos_c])
    full = np.concatenate(parts, axis=0)[:n]
    return np.ascontiguousarray(full, dtype=np.float32), res


def kernel(x, src, dst, W, b, gamma, beta):
    cfg, in_maps = host_prep(x, src, dst, W, b, gamma, beta)
    out, _ = run(in_maps, cfg)
    return out


# revision 21
# speedup vs baseline: 1.1351x; 1.0568x over previous
"""DeepGCNLayer (GraphConv norm='both' + BatchNorm + ReLU + residual) on 8 trn2 cores.

Sharding: nodes padded to NPAD=100352, split into 8 ranges (98 node-tiles of
128 per core). Edges routed to the core owning their dst (dst-sorted), then
per (dst-tile, src-window) padded to a uniform K_w chunks of 128 so every
core runs one SPMD program.

v2 layout: the AllGather'd z table is built in 4 row-interleaved windows
(each window = the same quarter of every core's slice) so per-window
AllGathers overlap the z pass and the first gather groups. The one-hot S
matrices are built as ACT-engine broadcast expansion of eloc plus a DVE
is_equal on real tiles (2x perf mode, short shared-port holds) - the v1
broadcast tensor_tensor held the DVE shared SBUF port for ~28us/group,
starving the SWDGE gather descriptor generator (see trainium-docs
memories/01-sbuf.md "DVE blocks DMA" trap). x ships transposed bf16 so the
z pass needs no PE transposes; norm_src/norm_dst fold into ACT scale-copies.
"""

import sys

if "/opt/trn_rl_repo" not in sys.path:
    sys.path.insert(0, "/opt/trn_rl_repo")

import numpy as np

P = 128
D = 128
NCORES = 8
BN_EPS = 1e-5
GROUP = 7            # dst tiles per gather group
PBC = 7              # node tiles per phase-B load/store batch
WTILES = (22, 27, 27, 22)   # z-table window sizes in node tiles (per core)

_NC_CACHE = {}


def build_program(npad, nt, kws, n_real):
    """kws: tuple of chunks-per-window per dst tile (uniform across tiles)."""
    import concourse.bacc as bacc
    import concourse.tile as tile
    from concourse import mybir

    f32 = mybir.dt.float32
    bf16 = mybir.dt.bfloat16
    i32 = mybir.dt.int32
    i16 = mybir.dt.int16
    OP = mybir.AluOpType
    AF = mybir.ActivationFunctionType

    nodes_pc = nt * P
    ktot = sum(kws)
    nwin = len(kws)
    ngroups = nt // GROUP
    assert nt % GROUP == 0
    assert sum(WTILES) == nt and len(WTILES) == nwin
    kmax = max(kws)
    # idx16 columns per (group, window); eloc columns per (group, window)
    cols_gw = [GROUP * kw * P // 16 for kw in kws]
    gcols_i = sum(cols_gw)              # idx cols per group
    gcols_e = GROUP * ktot              # eloc cols per group

    nc = bacc.Bacc("TRN2", target_bir_lowering=False, debug=False,
                   num_devices=NCORES, num_swdge_queues=4)

    xt = nc.dram_tensor("xt", [P, nodes_pc], bf16, kind="ExternalInput")
    xres = nc.dram_tensor("xres", [nodes_pc, D], f32, kind="ExternalInput")
    wmat = nc.dram_tensor("wmat", [D, D], bf16, kind="ExternalInput")
    grow = nc.dram_tensor("grow", [1, D], f32, kind="ExternalInput")
    brow = nc.dram_tensor("brow", [1, D], f32, kind="ExternalInput")
    dgo = nc.dram_tensor("dgo", [P, nt], i32, kind="ExternalInput")
    dgi = nc.dram_tensor("dgi", [P, nt], i32, kind="ExternalInput")
    idxs = nc.dram_tensor("idxs", [P, ngroups * gcols_i], i16,
                          kind="ExternalInput")
    eloc = nc.dram_tensor("eloc", [P, ngroups * gcols_e], bf16,
                          kind="ExternalInput")
    out = nc.dram_tensor("out", [nodes_pc, D], f32, kind="ExternalOutput")
    h_d = nc.dram_tensor("h_d", [nodes_pc, D], bf16)

    # per-window z contribution + AllGather'd table (row-interleaved:
    # window w = [core0 quarter w | core1 quarter w | ...])
    cc_z = [nc.dram_tensor(f"cc_z{w}", [WTILES[w] * P, D], bf16)
            for w in range(nwin)]
    z_t = [nc.dram_tensor(f"z_t{w}", [WTILES[w] * P * NCORES, D], bf16,
                          addr_space="Shared")
           for w in range(nwin)]

    with tile.TileContext(nc) as tc:
        with (
            tc.tile_pool(name="const", bufs=1) as constp,
            tc.tile_pool(name="norm", bufs=1) as normp,
            tc.tile_pool(name="xz", bufs=1) as xzp,
            tc.tile_pool(name="zst", bufs=1) as zsp,
            tc.tile_pool(name="meta", bufs=2) as metap,
            tc.tile_pool(name="erep", bufs=1) as erepp,
            tc.tile_pool(name="gathA", bufs=3) as gathA,
            tc.tile_pool(name="gathB", bufs=3) as gathB,
            tc.tile_pool(name="s", bufs=1) as sp,
            tc.tile_pool(name="work", bufs=2) as workp,
            tc.tile_pool(name="stats", bufs=1) as statp,
            tc.tile_pool(name="io", bufs=3) as iop,
            tc.tile_pool(name="psA", bufs=2, space="PSUM") as psA,
            tc.tile_pool(name="psB", bufs=2, space="PSUM") as psB,
            tc.tile_pool(name="psS", bufs=1, space="PSUM") as psS,
            tc.tile_pool(name="dram", bufs=2, space="DRAM") as dramp,
        ):
            # ---- constants -------------------------------------------------
            iota = constp.tile([P, P], bf16, tag="iota")
            nc.gpsimd.iota(iota[:], pattern=[[1, P]], base=0,
                           channel_multiplier=0,
                           allow_small_or_imprecise_dtypes=True)
            # iota replicated along free dim for batched 2x is_equal
            iota_rep = constp.tile([P, GROUP * kmax * P], bf16, tag="iotar")
            nc.vector.tensor_copy(
                iota_rep[:].rearrange("p (c e) -> p c e", e=P),
                iota[:, None, :].to_broadcast([P, GROUP * kmax, P]))
            ones1 = constp.tile([1, P], f32, tag="ones1")
            nc.vector.memset(ones1[:], 1.0)
            ones_c = constp.tile([P, 1], bf16, tag="ones_c")
            nc.vector.memset(ones_c[:], 1.0)
            w_sb = constp.tile([P, D], bf16, tag="wsb")
            nc.sync.dma_start(out=w_sb[:], in_=wmat[:])
            g_row = constp.tile([1, D], f32, tag="grow")
            nc.sync.dma_start(out=g_row[:], in_=grow[:])
            be_row = constp.tile([1, D], f32, tag="berow")
            nc.sync.dma_start(out=be_row[:], in_=brow[:])

            # ---- norm arrays (own range, F-order [P, nt]) ------------------
            deg = normp.tile([P, nt], i32, tag="deg")
            nc.sync.dma_start(out=deg[:], in_=dgo[:])
            degf = normp.tile([P, nt], f32, tag="degf")
            nc.vector.tensor_scalar_max(degf[:], deg[:], 1.0)
            nc.scalar.sqrt(degf[:], degf[:])
            ns_f = constp.tile([P, nt], f32, tag="ns_f")
            nc.vector.reciprocal(ns_f[:], degf[:])

            deg2 = normp.tile([P, nt], i32, tag="deg2")
            nc.sync.dma_start(out=deg2[:], in_=dgi[:])
            deg2f = normp.tile([P, nt], f32, tag="deg2f")
            nc.vector.tensor_scalar_max(deg2f[:], deg2[:], 1.0)
            nc.scalar.sqrt(deg2f[:], deg2f[:])
            nd_f = constp.tile([P, nt], f32, tag="nd_f")
            nc.vector.reciprocal(nd_f[:], deg2f[:])

            # ---- z pass: z = (x@W)*ns -> bf16, own slice, 4 windows --------
            # lhsT = xT slice (no transposes); ns folds into the ACT
            # PSUM->SBUF copy; each window's AllGather issues as soon as its
            # quarter is stored so window-0 gathers start early.
            t0 = 0
            for w in range(nwin):
                wt = WTILES[w]
                xt_w = xzp.tile([P, wt * P], bf16, tag="xt_w")
                nc.sync.dma_start(out=xt_w[:],
                                  in_=xt[:, t0 * P:(t0 + wt) * P])
                z_w = zsp.tile([P, wt * D], bf16, tag="z_w")
                for c in range(wt):
                    z_ps = psB.tile([P, D], f32, tag="B")
                    nc.tensor.matmul(out=z_ps[:],
                                     lhsT=xt_w[:, c * P:(c + 1) * P],
                                     rhs=w_sb[:], start=True, stop=True)
                    nc.scalar.activation(
                        out=z_w[:, c * D:(c + 1) * D], in_=z_ps[:],
                        func=AF.Copy, scale=ns_f[:, t0 + c:t0 + c + 1])
                nc.scalar.dma_start(
                    out=cc_z[w][:].rearrange("(c p) e -> p c e", p=P),
                    in_=z_w[:].rearrange("p (c e) -> p c e", e=D))
                nc.gpsimd.collective_compute(
                    "AllGather", OP.bypass,
                    replica_groups=[list(range(NCORES))],
                    ins=[cc_z[w][:]], outs=[z_t[w][:]])
                t0 += wt

            # ---- phase A ---------------------------------------------------
            sum_ps = psS.tile([1, P], f32, tag="sum")
            sq_ps = psS.tile([1, P], f32, tag="sq")

            for g in range(ngroups):
                idx_g = metap.tile([P, gcols_i], i16, tag="idxg")
                nc.sync.dma_start(
                    out=idx_g[:], in_=idxs[:, g * gcols_i:(g + 1) * gcols_i])
                eloc_g = metap.tile([P, gcols_e], bf16, tag="elocg")
                nc.sync.dma_start(
                    out=eloc_g[:], in_=eloc[:, g * gcols_e:(g + 1) * gcols_e])

                e_ws = []
                s_ws = []
                ico = 0
                eco = 0
                for w in range(nwin):
                    kw = kws[w]
                    nch = GROUP * kw
                    nidx = nch * P
                    pool = gathA if w < 2 else gathB
                    e_t = pool.tile([P, nch * D], bf16, tag=f"E{w}")
                    nc.gpsimd.dma_gather(
                        e_t[:].rearrange("p (c e) -> p c e", e=D),
                        z_t[w][:],
                        idx_g[:, ico:ico + nidx // 16],
                        nidx, nidx, D, single_packet=False,
                        queue_num=(g + w) % 4)
                    ico += nidx // 16
                    e_ws.append(e_t)
                    # eloc_rep: ACT broadcast expansion (own SBUF port),
                    # then one batched 2x is_equal on real tiles (short
                    # shared-port hold).
                    el_r = erepp.tile([P, nch * P], bf16, tag="R")
                    nc.scalar.copy(
                        el_r[:].rearrange("p (c e) -> p c e", e=P),
                        eloc_g[:, eco:eco + nch, None].to_broadcast(
                            [P, nch, P]))
                    s_t = sp.tile([P, nch * P], bf16, tag=f"S{w}")
                    nc.vector.tensor_tensor(
                        out=s_t[:], in0=el_r[:],
                        in1=iota_rep[:, :nch * P], op=OP.is_equal)
                    eco += nch
                    s_ws.append(s_t)

                h_g = workp.tile([P, GROUP * D], bf16, tag="hg")
                for u in range(GROUP):
                    t = g * GROUP + u
                    agg_ps = psA.tile([P, P], f32, tag="A")
                    ci = 0
                    for w in range(nwin):
                        kw = kws[w]
                        for j in range(kw):
                            nc.tensor.matmul(
                                out=agg_ps[:],
                                lhsT=s_ws[w][:, (u * kw + j) * P:
                                             (u * kw + j + 1) * P],
                                rhs=e_ws[w][:, (u * kw + j) * D:
                                            (u * kw + j + 1) * D],
                                start=(ci + j == 0),
                                stop=(ci + j == ktot - 1))
                        ci += kw
                    h_t = h_g[:, u * D:(u + 1) * D]
                    nc.scalar.activation(out=h_t, in_=agg_ps[:],
                                         func=AF.Copy,
                                         scale=nd_f[:, t:t + 1])
                    sq_sb = workp.tile([P, D], bf16, tag="sqsb")
                    nc.scalar.activation(out=sq_sb[:], in_=h_t,
                                         func=AF.Square)
                    nc.tensor.matmul(out=sum_ps[:], lhsT=ones_c[:], rhs=h_t,
                                     start=(t == 0), stop=(t == nt - 1))
                    nc.tensor.matmul(out=sq_ps[:], lhsT=ones_c[:],
                                     rhs=sq_sb[:],
                                     start=(t == 0), stop=(t == nt - 1))
                nc.scalar.dma_start(
                    out=h_d[:].rearrange("(c p) e -> p c e", p=P)[
                        :, g * GROUP:(g + 1) * GROUP, :],
                    in_=h_g[:].rearrange("p (c e) -> p c e", e=D))

            # ---- BN stats all-reduce + scale/shift (row layout) ------------
            srow = statp.tile([1, 2 * P], f32, tag="srow")
            nc.scalar.copy(srow[0:1, 0:P], sum_ps[:])
            nc.scalar.copy(srow[0:1, P:2 * P], sq_ps[:])
            cc_in = dramp.tile([1, 2 * P], f32, tag="ccin")
            cc_out = dramp.tile([1, 2 * P], f32, tag="ccout")
            nc.gpsimd.dma_start(out=cc_in[:], in_=srow[:])
            nc.gpsimd.collective_compute(
                "AllReduce", OP.add,
                replica_groups=[list(range(NCORES))],
                ins=[cc_in.opt()], outs=[cc_out.opt()])
            grow_sb = statp.tile([1, 2 * P], f32, tag="grow_sb")
            nc.gpsimd.dma_start(out=grow_sb[:], in_=cc_out[:])

            inv_n = 1.0 / float(n_real)
            mean_r = statp.tile([1, P], f32, tag="mean")
            nc.vector.tensor_scalar_mul(mean_r[:], grow_sb[0:1, 0:P], inv_n)
            ex2_r = statp.tile([1, P], f32, tag="ex2")
            nc.vector.tensor_scalar_mul(ex2_r[:], grow_sb[0:1, P:2 * P],
                                        inv_n)
            m2_r = statp.tile([1, P], f32, tag="m2")
            nc.scalar.square(m2_r[:], mean_r[:])
            var_r = statp.tile([1, P], f32, tag="var")
            nc.vector.tensor_tensor(out=var_r[:], in0=ex2_r[:], in1=m2_r[:],
                                    op=OP.subtract)
            nc.vector.tensor_scalar_add(var_r[:], var_r[:], BN_EPS)
            sd_r = statp.tile([1, P], f32, tag="sd")
            nc.scalar.sqrt(sd_r[:], var_r[:])
            inv_r = statp.tile([1, P], f32, tag="inv")
            nc.vector.reciprocal(inv_r[:], sd_r[:])
            sc_r = statp.tile([1, P], f32, tag="sc")
            nc.vector.tensor_tensor(out=sc_r[:], in0=g_row[:], in1=inv_r[:],
                                    op=OP.mult)
            # b cancels in BN: shift = beta - mean*scale
            tc_r = statp.tile([1, P], f32, tag="tc")
            nc.vector.tensor_tensor(out=tc_r[:], in0=mean_r[:], in1=sc_r[:],
                                    op=OP.mult)
            nc.vector.tensor_tensor(out=tc_r[:], in0=be_row[:], in1=tc_r[:],
                                    op=OP.subtract)

            # rank-1 broadcast of sc/tc across partitions -> bf16 tiles
            scb_ps = psA.tile([P, P], f32, tag="A")
            nc.tensor.matmul(out=scb_ps[:], lhsT=ones1[:], rhs=sc_r[:],
                             start=True, stop=True)
            sc_bc = constp.tile([P, P], bf16, tag="sc_bc")
            nc.scalar.copy(sc_bc[:], scb_ps[:])
            tcb_ps = psB.tile([P, P], f32, tag="B")
            nc.tensor.matmul(out=tcb_ps[:], lhsT=ones1[:], rhs=tc_r[:],
                             start=True, stop=True)
            tc_bc = constp.tile([P, P], bf16, tag="tc_bc")
            nc.scalar.copy(tc_bc[:], tcb_ps[:])

            # ---- phase B (batched loads/stores, no transposes) -------------
            for bt in range(nt // PBC):
                t0 = bt * PBC
                x_b = iop.tile([P, PBC * D], f32, tag="xb")
                nc.scalar.dma_start(
                    out=x_b[:].rearrange("p (c e) -> p c e", e=D),
                    in_=xres[:].rearrange("(c p) e -> p c e", p=P)[
                        :, t0:t0 + PBC, :])
                h_b = iop.tile([P, PBC * D], bf16, tag="hb")
                nc.sync.dma_start(
                    out=h_b[:].rearrange("p (c e) -> p c e", e=D),
                    in_=h_d[:].rearrange("(c p) e -> p c e", p=P)[
                        :, t0:t0 + PBC, :])
                g1 = workp.tile([P, PBC * D], bf16, tag="g1")
                nc.vector.tensor_tensor(
                    out=g1[:].rearrange("p (c e) -> p c e", e=D),
                    in0=h_b[:].rearrange("p (c e) -> p c e", e=D),
                    in1=sc_bc[:, None, :].to_broadcast([P, PBC, D]),
                    op=OP.mult)
                nc.vector.tensor_tensor(
                    out=g1[:].rearrange("p (c e) -> p c e", e=D),
                    in0=g1[:].rearrange("p (c e) -> p c e", e=D),
                    in1=tc_bc[:, None, :].to_broadcast([P, PBC, D]),
                    op=OP.add)
                nc.vector.tensor_scalar_max(g1[:], g1[:], 0.0)
                nc.vector.tensor_tensor(out=x_b[:], in0=g1[:], in1=x_b[:],
                                        op=OP.add)
                nc.sync.dma_start(
                    out=out[:].rearrange("(c p) e -> p c e", p=P)[
                        :, t0:t0 + PBC, :],
                    in_=x_b[:].rearrange("p (c e) -> p c e", e=D))

    nc.compile()
    return nc


def _wrap16(a):
    b = a.reshape(-1, 16).T
    return np.tile(b, (8, 1))


def host_prep(x, src, dst, W, b, gamma, beta):
    """Graph routing / layout prep (indices only - no FLOPs on host)."""
    import ml_dtypes

    x = np.asarray(x, np.float32)
    W = np.asarray(W, np.float32)
    gamma = np.asarray(gamma, np.float32)
    beta = np.asarray(beta, np.float32)
    src32 = np.asarray(src).astype(np.int64)
    dst32 = np.asarray(dst).astype(np.int64)

    n = x.shape[0]
    npad = -(-n // (P * NCORES * GROUP)) * (P * NCORES * GROUP)
    nodes_pc = npad // NCORES
    nt = nodes_pc // P
    nt_tot = npad // P
    assert sum(WTILES) == nt
    nwin = len(WTILES)
    wt_start = np.cumsum([0] + list(WTILES))  # in tiles, per core

    order = np.argsort(dst32, kind="stable")
    ds = dst32[order]
    ss = src32[order]

    ar = np.arange(npad + 1, dtype=np.int64)
    rps = np.searchsorted(np.sort(src32), ar).astype(np.int32)
    rpd_full = np.searchsorted(ds, ar)

    # src -> (window, row within window table). Window w of the z table is
    # [core0 quarter w | core1 quarter w | ...], quarter w = tiles
    # [wt_start[w], wt_start[w+1]) of each core's slice.
    s_core = ss // nodes_pc
    s_r = ss % nodes_pc
    s_tile = s_r // P
    s_win = np.searchsorted(wt_start, s_tile, side="right") - 1
    wrows = (np.array(WTILES) * P)[s_win]
    s_winrow = s_core * wrows + (s_r - wt_start[s_win] * P)

    # degree counts (int), F-order [P, nt] per core
    dgo_n = np.diff(rps).astype(np.int32)                 # [npad]
    dgi_n = np.diff(rpd_full).astype(np.int32)            # [npad]

    # per-dst in-degree split by src window
    deg4 = np.zeros((npad, nwin), np.int32)
    np.add.at(deg4, (ds, s_win), 1)

    # --- bin-pack dst nodes into tiles (per core) to flatten the
    # per-(tile, window) edge-count tails, so kws (chunk counts) shrink.
    caps = np.array([4 * P, 5 * P, 5 * P, 4 * P], np.float64)
    newpos = np.empty(npad, np.int64)    # global node -> permuted local slot
    for c in range(NCORES):
        d4 = deg4[c * nodes_pc:(c + 1) * nodes_pc].astype(np.float64)
        order_c = np.argsort(-d4.sum(1), kind="stable")
        loads = np.zeros((nt, nwin))
        counts = np.zeros(nt, np.int64)
        fill = [[] for _ in range(nt)]
        for i in order_c:
            util = np.max((loads + d4[i]) / caps, axis=1)
            util[counts >= P] = np.inf
            b = int(np.argmin(util))
            loads[b] += d4[i]
            counts[b] += 1
            fill[b].append(i)
        pos = np.empty(nodes_pc, np.int64)
        for b in range(nt):
            pos[np.array(fill[b], np.int64)] = (
                b * P + np.arange(len(fill[b])))
        newpos[c * nodes_pc:(c + 1) * nodes_pc] = pos

    # kws from the packed loads (global max over cores/tiles per window)
    e_core = ds // nodes_pc
    e_pos = newpos[ds]
    e_tile = e_core * nt + e_pos // P
    e_slot = e_pos % P
    cell = e_tile * nwin + s_win
    cnt = np.bincount(cell, minlength=nt_tot * nwin).reshape(nt_tot, nwin)
    kws = tuple(int(-(-cnt[:, w].max() // P)) for w in range(nwin))
    ktot = sum(kws)

    # per (tile, window) edge lists under the permutation
    eorder = np.argsort(cell, kind="stable")
    bnd = np.searchsorted(cell[eorder], np.arange(nt_tot * nwin + 1))
    tw_lists = [[eorder[bnd[t * nwin + w]:bnd[t * nwin + w + 1]]
                 for w in range(nwin)] for t in range(nt_tot)]

    xpad = np.zeros((npad, D), np.float32)
    xpad[:n] = x

    ngroups = nt // GROUP
    in_maps = []
    shared = dict(
        wmat=W.astype(ml_dtypes.bfloat16),
        grow=np.ascontiguousarray(gamma[None, :]),
        brow=np.ascontiguousarray(beta[None, :]))
    for c in range(NCORES):
        # eloc layout: (g, w, u, chunk) contiguous for batched expansions
        elocv = np.full((nt * ktot, P), -1.0, np.float32)
        idx_blocks = []
        ecol_off = 0
        for g in range(ngroups):
            for w in range(nwin):
                blk = np.zeros(GROUP * kws[w] * P, np.int16)
                for u in range(GROUP):
                    t = g * GROUP + u
                    gt = c * nt + t
                    sel = tw_lists[gt][w]
                    base = u * kws[w] * P
                    blk[base:base + len(sel)] = s_winrow[sel].astype(np.int16)
                    ev = e_slot[sel].astype(np.float32)
                    ecol = elocv[ecol_off + u * kws[w]:
                                 ecol_off + (u + 1) * kws[w]].reshape(-1)
                    ecol[:len(sel)] = ev
                ecol_off += GROUP * kws[w]
                idx_blocks.append(_wrap16(blk))
        m = dict(shared)
        xslice = xpad[c * nodes_pc:(c + 1) * nodes_pc]
        pos_c = newpos[c * nodes_pc:(c + 1) * nodes_pc]
        orig_of = np.empty(nodes_pc, np.int64)
        orig_of[pos_c] = np.arange(nodes_pc)
        m["xt"] = np.ascontiguousarray(xslice.T).astype(ml_dtypes.bfloat16)
        m["xres"] = np.ascontiguousarray(xslice[orig_of])
        m["dgo"] = np.ascontiguousarray(
            dgo_n[c * nodes_pc:(c + 1) * nodes_pc].reshape(nt, P).T)
        m["dgi"] = np.ascontiguousarray(
            dgi_n[c * nodes_pc:(c + 1) * nodes_pc][orig_of]
            .reshape(nt, P).T)
        m["idxs"] = np.ascontiguousarray(np.concatenate(idx_blocks, axis=1))
        # eloc device layout: col (g,w,u,chunk) partition p = edge c*128+p
        m["eloc"] = np.ascontiguousarray(
            elocv.T).astype(ml_dtypes.bfloat16)
        in_maps.append(m)
    return dict(npad=npad, nt=nt, kws=kws, n_real=n, newpos=newpos), in_maps


def run(in_maps, cfg, **kw):
    from concourse.bass_utils import run_bass_kernel_spmd

    key = (cfg["npad"], cfg["nt"], tuple(cfg["kws"]), cfg["n_real"])
    if key not in _NC_CACHE:
        _NC_CACHE[key] = build_program(*key)
    nc = _NC_CACHE[key]
    res = run_bass_kernel_spmd(nc, in_maps, core_ids=list(range(NCORES)), **kw)
    n = cfg["n_real"]
    nodes_pc = cfg["npad"] // NCORES
    parts = []
    for c in range(NCORES):
        o = np.asarray(res.results[c]["out"])
        pos_c = cfg["newpos"][c * nodes_pc:(c + 1) * nodes_pc]
        parts.append(o[pos_c])
    full = np.concatenate(parts, axis=0)[:n]
    return np.ascontiguousarray(full, dtype=np.float32), res


def kernel(x, src, dst, W, b, gamma, beta):
    cfg, in_maps = host_prep(x, src, dst, W, b, gamma, beta)
    out, _ = run(in_maps, cfg)
    return out


# revision 23
# speedup vs baseline: 1.2231x; 1.0775x over previous
"""DeepGCNLayer (GraphConv norm='both' + BatchNorm + ReLU + residual) on 8 trn2 cores.

Sharding: nodes padded to NPAD=100352, split into 8 ranges (98 node-tiles of
128 per core). Edges routed to the core owning their dst (dst-sorted), then
per (dst-tile, src-window) padded to a uniform K_w chunks of 128 so every
core runs one SPMD program.

v2 layout: the AllGather'd z table is built in 4 row-interleaved windows
(each window = the same quarter of every core's slice) so per-window
AllGathers overlap the z pass and the first gather groups. The one-hot S
matrices are built as ACT-engine broadcast expansion of eloc plus a DVE
is_equal on real tiles (2x perf mode, short shared-port holds) - the v1
broadcast tensor_tensor held the DVE shared SBUF port for ~28us/group,
starving the SWDGE gather descriptor generator (see trainium-docs
memories/01-sbuf.md "DVE blocks DMA" trap). x ships transposed bf16 so the
z pass needs no PE transposes; norm_src/norm_dst fold into ACT scale-copies.
"""

import sys

if "/opt/trn_rl_repo" not in sys.path:
    sys.path.insert(0, "/opt/trn_rl_repo")

import numpy as np

P = 128
D = 128
NCORES = 8
BN_EPS = 1e-5
GROUP = 7            # dst tiles per gather group
PBC = 7              # node tiles per phase-B load/store batch
WTILES = (22, 27, 27, 22)   # z-table window sizes in node tiles (per core)

_NC_CACHE = {}


def build_program(npad, nt, kws, n_real):
    """kws: tuple of chunks-per-window per dst tile (uniform across tiles)."""
    import concourse.bacc as bacc
    import concourse.tile as tile
    from concourse import mybir

    f32 = mybir.dt.float32
    bf16 = mybir.dt.bfloat16
    i32 = mybir.dt.int32
    i16 = mybir.dt.int16
    OP = mybir.AluOpType
    AF = mybir.ActivationFunctionType

    nodes_pc = nt * P
    ktot = sum(kws)
    nwin = len(kws)
    ngroups = nt // GROUP
    assert nt % GROUP == 0
    assert sum(WTILES) == nt and len(WTILES) == nwin
    kmax = max(kws)
    # idx16 columns per (group, window); eloc columns per (group, window)
    cols_gw = [GROUP * kw * P // 16 for kw in kws]
    gcols_i = sum(cols_gw)              # idx cols per group
    gcols_e = GROUP * ktot              # eloc cols per group

    nc = bacc.Bacc("TRN2", target_bir_lowering=False, debug=False,
                   num_devices=NCORES, num_swdge_queues=4)

    xt = nc.dram_tensor("xt", [P, nodes_pc], bf16, kind="ExternalInput")
    xres = nc.dram_tensor("xres", [nodes_pc, D], f32, kind="ExternalInput")
    wmat = nc.dram_tensor("wmat", [D, D], bf16, kind="ExternalInput")
    grow = nc.dram_tensor("grow", [1, D], f32, kind="ExternalInput")
    brow = nc.dram_tensor("brow", [1, D], f32, kind="ExternalInput")
    dgo = nc.dram_tensor("dgo", [P, nt], i32, kind="ExternalInput")
    dgi = nc.dram_tensor("dgi", [P, nt], i32, kind="ExternalInput")
    idxs = nc.dram_tensor("idxs", [P, ngroups * gcols_i], i16,
                          kind="ExternalInput")
    eloc = nc.dram_tensor("eloc", [P, ngroups * gcols_e], bf16,
                          kind="ExternalInput")
    out = nc.dram_tensor("out", [nodes_pc, D], f32, kind="ExternalOutput")
    h_d = nc.dram_tensor("h_d", [nodes_pc, D], bf16)

    # per-window z contribution + AllGather'd table (row-interleaved:
    # window w = [core0 quarter w | core1 quarter w | ...])
    cc_z = [nc.dram_tensor(f"cc_z{w}", [WTILES[w] * P, D], bf16)
            for w in range(nwin)]
    z_t = [nc.dram_tensor(f"z_t{w}", [WTILES[w] * P * NCORES, D], bf16,
                          addr_space="Shared")
           for w in range(nwin)]

    with tile.TileContext(nc) as tc:
        with (
            tc.tile_pool(name="const", bufs=1) as constp,
            tc.tile_pool(name="norm", bufs=1) as normp,
            tc.tile_pool(name="xz", bufs=1) as xzp,
            tc.tile_pool(name="zst", bufs=1) as zsp,
            tc.tile_pool(name="meta", bufs=2) as metap,
            tc.tile_pool(name="gathA", bufs=3) as gathA,
            tc.tile_pool(name="gathB", bufs=3) as gathB,
            tc.tile_pool(name="s", bufs=1) as sp,
            tc.tile_pool(name="work", bufs=2) as workp,
            tc.tile_pool(name="stats", bufs=1) as statp,
            tc.tile_pool(name="io", bufs=3) as iop,
            tc.tile_pool(name="psA", bufs=2, space="PSUM") as psA,
            tc.tile_pool(name="psB", bufs=2, space="PSUM") as psB,
            tc.tile_pool(name="psS", bufs=1, space="PSUM") as psS,
            tc.tile_pool(name="dram", bufs=2, space="DRAM") as dramp,
        ):
            # ---- constants -------------------------------------------------
            iota = constp.tile([P, P], bf16, tag="iota")
            nc.gpsimd.iota(iota[:], pattern=[[1, P]], base=0,
                           channel_multiplier=0,
                           allow_small_or_imprecise_dtypes=True)
            # iota replicated along free dim for batched 2x is_equal
            iota_rep = constp.tile([P, GROUP * kmax * P], bf16, tag="iotar")
            nc.vector.tensor_copy(
                iota_rep[:].rearrange("p (c e) -> p c e", e=P),
                iota[:, None, :].to_broadcast([P, GROUP * kmax, P]))
            ones1 = constp.tile([1, P], f32, tag="ones1")
            nc.vector.memset(ones1[:], 1.0)
            ones_c = constp.tile([P, 1], bf16, tag="ones_c")
            nc.vector.memset(ones_c[:], 1.0)
            w_sb = constp.tile([P, D], bf16, tag="wsb")
            nc.sync.dma_start(out=w_sb[:], in_=wmat[:])
            g_row = constp.tile([1, D], f32, tag="grow")
            nc.sync.dma_start(out=g_row[:], in_=grow[:])
            be_row = constp.tile([1, D], f32, tag="berow")
            nc.sync.dma_start(out=be_row[:], in_=brow[:])

            # ---- norm arrays (own range, F-order [P, nt]) ------------------
            deg = normp.tile([P, nt], i32, tag="deg")
            nc.sync.dma_start(out=deg[:], in_=dgo[:])
            degf = normp.tile([P, nt], f32, tag="degf")
            nc.vector.tensor_scalar_max(degf[:], deg[:], 1.0)
            nc.scalar.sqrt(degf[:], degf[:])
            ns_f = constp.tile([P, nt], f32, tag="ns_f")
            nc.vector.reciprocal(ns_f[:], degf[:])

            deg2 = normp.tile([P, nt], i32, tag="deg2")
            nc.sync.dma_start(out=deg2[:], in_=dgi[:])
            deg2f = normp.tile([P, nt], f32, tag="deg2f")
            nc.vector.tensor_scalar_max(deg2f[:], deg2[:], 1.0)
            nc.scalar.sqrt(deg2f[:], deg2f[:])
            nd_f = constp.tile([P, nt], f32, tag="nd_f")
            nc.vector.reciprocal(nd_f[:], deg2f[:])

            # ---- z pass: z = (x@W)*ns -> bf16, own slice, 4 windows --------
            # lhsT = xT slice (no transposes); ns folds into the ACT
            # PSUM->SBUF copy; each window's AllGather issues as soon as its
            # quarter is stored so window-0 gathers start early.
            t0 = 0
            for w in range(nwin):
                wt = WTILES[w]
                xt_w = xzp.tile([P, wt * P], bf16, tag="xt_w")
                nc.sync.dma_start(out=xt_w[:],
                                  in_=xt[:, t0 * P:(t0 + wt) * P])
                z_w = zsp.tile([P, wt * D], bf16, tag="z_w")
                for c in range(wt):
                    z_ps = psB.tile([P, D], f32, tag="B")
                    nc.tensor.matmul(out=z_ps[:],
                                     lhsT=xt_w[:, c * P:(c + 1) * P],
                                     rhs=w_sb[:], start=True, stop=True)
                    nc.scalar.activation(
                        out=z_w[:, c * D:(c + 1) * D], in_=z_ps[:],
                        func=AF.Copy, scale=ns_f[:, t0 + c:t0 + c + 1])
                nc.scalar.dma_start(
                    out=cc_z[w][:].rearrange("(c p) e -> p c e", p=P),
                    in_=z_w[:].rearrange("p (c e) -> p c e", e=D))
                nc.gpsimd.collective_compute(
                    "AllGather", OP.bypass,
                    replica_groups=[list(range(NCORES))],
                    ins=[cc_z[w][:]], outs=[z_t[w][:]])
                t0 += wt

            # ---- phase A ---------------------------------------------------
            sum_ps = psS.tile([1, P], f32, tag="sum")
            sq_ps = psS.tile([1, P], f32, tag="sq")

            for g in range(ngroups):
                idx_g = metap.tile([P, gcols_i], i16, tag="idxg")
                nc.sync.dma_start(
                    out=idx_g[:], in_=idxs[:, g * gcols_i:(g + 1) * gcols_i])
                eloc_g = metap.tile([P, gcols_e], bf16, tag="elocg")
                nc.sync.dma_start(
                    out=eloc_g[:], in_=eloc[:, g * gcols_e:(g + 1) * gcols_e])

                e_ws = []
                s_ws = []
                ico = 0
                eco = 0
                for w in range(nwin):
                    kw = kws[w]
                    nch = GROUP * kw
                    nidx = nch * P
                    pool = gathA if w < 2 else gathB
                    e_t = pool.tile([P, nch * D], bf16, tag=f"E{w}")
                    nc.gpsimd.dma_gather(
                        e_t[:].rearrange("p (c e) -> p c e", e=D),
                        z_t[w][:],
                        idx_g[:, ico:ico + nidx // 16],
                        nidx, nidx, D, single_packet=False,
                        queue_num=(g + w) % 4)
                    ico += nidx // 16
                    e_ws.append(e_t)
                    s_t = sp.tile([P, nch * P], bf16, tag=f"S{w}")
                    nc.vector.tensor_tensor(
                        out=s_t[:].rearrange("p (c e) -> p c e", e=P),
                        in0=eloc_g[:, eco:eco + nch, None].to_broadcast(
                            [P, nch, P]),
                        in1=iota_rep[:, :nch * P].rearrange(
                            "p (c e) -> p c e", e=P),
                        op=OP.is_equal)
                    eco += nch
                    s_ws.append(s_t)

                h_g = workp.tile([P, GROUP * D], bf16, tag="hg")
                for u in range(GROUP):
                    t = g * GROUP + u
                    agg_ps = psA.tile([P, P], f32, tag="A")
                    ci = 0
                    for w in range(nwin):
                        kw = kws[w]
                        for j in range(kw):
                            nc.tensor.matmul(
                                out=agg_ps[:],
                                lhsT=s_ws[w][:, (u * kw + j) * P:
                                             (u * kw + j + 1) * P],
                                rhs=e_ws[w][:, (u * kw + j) * D:
                                            (u * kw + j + 1) * D],
                                start=(ci + j == 0),
                                stop=(ci + j == ktot - 1))
                        ci += kw
                    h_t = h_g[:, u * D:(u + 1) * D]
                    nc.scalar.activation(out=h_t, in_=agg_ps[:],
                                         func=AF.Copy,
                                         scale=nd_f[:, t:t + 1])
                    sq_sb = workp.tile([P, D], bf16, tag="sqsb")
                    nc.scalar.activation(out=sq_sb[:], in_=h_t,
                                         func=AF.Square)
                    nc.tensor.matmul(out=sum_ps[:], lhsT=ones_c[:], rhs=h_t,
                                     start=(t == 0), stop=(t == nt - 1))
                    nc.tensor.matmul(out=sq_ps[:], lhsT=ones_c[:],
                                     rhs=sq_sb[:],
                                     start=(t == 0), stop=(t == nt - 1))
                nc.scalar.dma_start(
                    out=h_d[:].rearrange("(c p) e -> p c e", p=P)[
                        :, g * GROUP:(g + 1) * GROUP, :],
                    in_=h_g[:].rearrange("p (c e) -> p c e", e=D))

            # ---- BN stats all-reduce + scale/shift (row layout) ------------
            srow = statp.tile([1, 2 * P], f32, tag="srow")
            nc.scalar.copy(srow[0:1, 0:P], sum_ps[:])
            nc.scalar.copy(srow[0:1, P:2 * P], sq_ps[:])
            cc_in = dramp.tile([1, 2 * P], f32, tag="ccin")
            cc_out = dramp.tile([1, 2 * P], f32, tag="ccout")
            nc.gpsimd.dma_start(out=cc_in[:], in_=srow[:])
            nc.gpsimd.collective_compute(
                "AllReduce", OP.add,
                replica_groups=[list(range(NCORES))],
                ins=[cc_in.opt()], outs=[cc_out.opt()])
            grow_sb = statp.tile([1, 2 * P], f32, tag="grow_sb")
            nc.gpsimd.dma_start(out=grow_sb[:], in_=cc_out[:])

            inv_n = 1.0 / float(n_real)
            mean_r = statp.tile([1, P], f32, tag="mean")
            nc.vector.tensor_scalar_mul(mean_r[:], grow_sb[0:1, 0:P], inv_n)
            ex2_r = statp.tile([1, P], f32, tag="ex2")
            nc.vector.tensor_scalar_mul(ex2_r[:], grow_sb[0:1, P:2 * P],
                                        inv_n)
            m2_r = statp.tile([1, P], f32, tag="m2")
            nc.scalar.square(m2_r[:], mean_r[:])
            var_r = statp.tile([1, P], f32, tag="var")
            nc.vector.tensor_tensor(out=var_r[:], in0=ex2_r[:], in1=m2_r[:],
                                    op=OP.subtract)
            nc.vector.tensor_scalar_add(var_r[:], var_r[:], BN_EPS)
            sd_r = statp.tile([1, P], f32, tag="sd")
            nc.scalar.sqrt(sd_r[:], var_r[:])
            inv_r = statp.tile([1, P], f32, tag="inv")
            nc.vector.reciprocal(inv_r[:], sd_r[:])
            sc_r = statp.tile([1, P], f32, tag="sc")
            nc.vector.tensor_tensor(out=sc_r[:], in0=g_row[:], in1=inv_r[:],
                                    op=OP.mult)
            # b cancels in BN: shift = beta - mean*scale
            tc_r = statp.tile([1, P], f32, tag="tc")
            nc.vector.tensor_tensor(out=tc_r[:], in0=mean_r[:], in1=sc_r[:],
                                    op=OP.mult)
            nc.vector.tensor_tensor(out=tc_r[:], in0=be_row[:], in1=tc_r[:],
                                    op=OP.subtract)

            # rank-1 broadcast of sc/tc across partitions -> bf16 tiles
            scb_ps = psA.tile([P, P], f32, tag="A")
            nc.tensor.matmul(out=scb_ps[:], lhsT=ones1[:], rhs=sc_r[:],
                             start=True, stop=True)
            sc_bc = constp.tile([P, P], bf16, tag="sc_bc")
            nc.scalar.copy(sc_bc[:], scb_ps[:])
            tcb_ps = psB.tile([P, P], f32, tag="B")
            nc.tensor.matmul(out=tcb_ps[:], lhsT=ones1[:], rhs=tc_r[:],
                             start=True, stop=True)
            tc_bc = constp.tile([P, P], bf16, tag="tc_bc")
            nc.scalar.copy(tc_bc[:], tcb_ps[:])

            # ---- phase B (batched loads/stores, no transposes) -------------
            for bt in range(nt // PBC):
                t0 = bt * PBC
                x_b = iop.tile([P, PBC * D], f32, tag="xb")
                nc.scalar.dma_start(
                    out=x_b[:].rearrange("p (c e) -> p c e", e=D),
                    in_=xres[:].rearrange("(c p) e -> p c e", p=P)[
                        :, t0:t0 + PBC, :])
                h_b = iop.tile([P, PBC * D], bf16, tag="hb")
                nc.sync.dma_start(
                    out=h_b[:].rearrange("p (c e) -> p c e", e=D),
                    in_=h_d[:].rearrange("(c p) e -> p c e", p=P)[
                        :, t0:t0 + PBC, :])
                g1 = workp.tile([P, PBC * D], bf16, tag="g1")
                nc.vector.tensor_tensor(
                    out=g1[:].rearrange("p (c e) -> p c e", e=D),
                    in0=h_b[:].rearrange("p (c e) -> p c e", e=D),
                    in1=sc_bc[:, None, :].to_broadcast([P, PBC, D]),
                    op=OP.mult)
                nc.vector.tensor_tensor(
                    out=g1[:].rearrange("p (c e) -> p c e", e=D),
                    in0=g1[:].rearrange("p (c e) -> p c e", e=D),
                    in1=tc_bc[:, None, :].to_broadcast([P, PBC, D]),
                    op=OP.add)
                nc.vector.tensor_scalar_max(g1[:], g1[:], 0.0)
                nc.vector.tensor_tensor(out=x_b[:], in0=g1[:], in1=x_b[:],
                                        op=OP.add)
                nc.sync.dma_start(
                    out=out[:].rearrange("(c p) e -> p c e", p=P)[
                        :, t0:t0 + PBC, :],
                    in_=x_b[:].rearrange("p (c e) -> p c e", e=D))

    nc.compile()
    return nc


def _wrap16(a):
    b = a.reshape(-1, 16).T
    return np.tile(b, (8, 1))


def host_prep(x, src, dst, W, b, gamma, beta):
    """Graph routing / layout prep (indices only - no FLOPs on host)."""
    import ml_dtypes

    x = np.asarray(x, np.float32)
    W = np.asarray(W, np.float32)
    gamma = np.asarray(gamma, np.float32)
    beta = np.asarray(beta, np.float32)
    src32 = np.asarray(src).astype(np.int64)
    dst32 = np.asarray(dst).astype(np.int64)

    n = x.shape[0]
    npad = -(-n // (P * NCORES * GROUP)) * (P * NCORES * GROUP)
    nodes_pc = npad // NCORES
    nt = nodes_pc // P
    nt_tot = npad // P
    assert sum(WTILES) == nt
    nwin = len(WTILES)
    wt_start = np.cumsum([0] + list(WTILES))  # in tiles, per core

    order = np.argsort(dst32, kind="stable")
    ds = dst32[order]
    ss = src32[order]

    ar = np.arange(npad + 1, dtype=np.int64)
    rps = np.searchsorted(np.sort(src32), ar).astype(np.int32)
    rpd_full = np.searchsorted(ds, ar)

    # src -> (window, row within window table). Window w of the z table is
    # [core0 quarter w | core1 quarter w | ...], quarter w = tiles
    # [wt_start[w], wt_start[w+1]) of each core's slice.
    s_core = ss // nodes_pc
    s_r = ss % nodes_pc
    s_tile = s_r // P
    s_win = np.searchsorted(wt_start, s_tile, side="right") - 1
    wrows = (np.array(WTILES) * P)[s_win]
    s_winrow = s_core * wrows + (s_r - wt_start[s_win] * P)

    # degree counts (int), F-order [P, nt] per core
    dgo_n = np.diff(rps).astype(np.int32)                 # [npad]
    dgi_n = np.diff(rpd_full).astype(np.int32)            # [npad]

    # per-dst in-degree split by src window
    deg4 = np.zeros((npad, nwin), np.int32)
    np.add.at(deg4, (ds, s_win), 1)

    # --- bin-pack dst nodes into tiles (per core) to flatten the
    # per-(tile, window) edge-count tails, so kws (chunk counts) shrink.
    caps = np.array([4 * P, 5 * P, 5 * P, 4 * P], np.float64)
    newpos = np.empty(npad, np.int64)    # global node -> permuted local slot
    for c in range(NCORES):
        d4 = deg4[c * nodes_pc:(c + 1) * nodes_pc].astype(np.float64)
        order_c = np.argsort(-d4.sum(1), kind="stable")
        loads = np.zeros((nt, nwin))
        counts = np.zeros(nt, np.int64)
        fill = [[] for _ in range(nt)]
        for i in order_c:
            util = np.max((loads + d4[i]) / caps, axis=1)
            util[counts >= P] = np.inf
            b = int(np.argmin(util))
            loads[b] += d4[i]
            counts[b] += 1
            fill[b].append(i)
        pos = np.empty(nodes_pc, np.int64)
        for b in range(nt):
            pos[np.array(fill[b], np.int64)] = (
                b * P + np.arange(len(fill[b])))
        newpos[c * nodes_pc:(c + 1) * nodes_pc] = pos

    # kws from the packed loads (global max over cores/tiles per window)
    e_core = ds // nodes_pc
    e_pos = newpos[ds]
    e_tile = e_core * nt + e_pos // P
    e_slot = e_pos % P
    cell = e_tile * nwin + s_win
    cnt = np.bincount(cell, minlength=nt_tot * nwin).reshape(nt_tot, nwin)
    kws = tuple(int(-(-cnt[:, w].max() // P)) for w in range(nwin))
    ktot = sum(kws)

    # per (tile, window) edge lists under the permutation
    eorder = np.argsort(cell, kind="stable")
    bnd = np.searchsorted(cell[eorder], np.arange(nt_tot * nwin + 1))
    tw_lists = [[eorder[bnd[t * nwin + w]:bnd[t * nwin + w + 1]]
                 for w in range(nwin)] for t in range(nt_tot)]

    xpad = np.zeros((npad, D), np.float32)
    xpad[:n] = x

    ngroups = nt // GROUP
    in_maps = []
    shared = dict(
        wmat=W.astype(ml_dtypes.bfloat16),
        grow=np.ascontiguousarray(gamma[None, :]),
        brow=np.ascontiguousarray(beta[None, :]))
    for c in range(NCORES):
        # eloc layout: (g, w, u, chunk) contiguous for batched expansions
        elocv = np.full((nt * ktot, P), -1.0, np.float32)
        idx_blocks = []
        ecol_off = 0
        for g in range(ngroups):
            for w in range(nwin):
                blk = np.zeros(GROUP * kws[w] * P, np.int16)
                for u in range(GROUP):
                    t = g * GROUP + u
                    gt = c * nt + t
                    sel = tw_lists[gt][w]
                    base = u * kws[w] * P
                    blk[base:base + len(sel)] = s_winrow[sel].astype(np.int16)
                    ev = e_slot[sel].astype(np.float32)
                    ecol = elocv[ecol_off + u * kws[w]:
                                 ecol_off + (u + 1) * kws[w]].reshape(-1)
                    ecol[:len(sel)] = ev
                ecol_off += GROUP * kws[w]
                idx_blocks.append(_wrap16(blk))
        m = dict(shared)
        xslice = xpad[c * nodes_pc:(c + 1) * nodes_pc]
        pos_c = newpos[c * nodes_pc:(c + 1) * nodes_pc]
        orig_of = np.empty(nodes_pc, np.int64)
        orig_of[pos_c] = np.arange(nodes_pc)
        m["xt"] = np.ascontiguousarray(xslice.T).astype(ml_dtypes.bfloat16)
        m["xres"] = np.ascontiguousarray(xslice[orig_of])
        m["dgo"] = np.ascontiguousarray(
            dgo_n[c * nodes_pc:(c + 1) * nodes_pc].reshape(nt, P).T)
        m["dgi"] = np.ascontiguousarray(
            dgi_n[c * nodes_pc:(c + 1) * nodes_pc][orig_of]
            .reshape(nt, P).T)
        m["idxs"] = np.ascontiguousarray(np.concatenate(idx_blocks, axis=1))
        # eloc device layout: col (g,w,u,chunk) partition p = edge c*128+p
        m["eloc"] = np.ascontiguousarray(
            elocv.T).astype(ml_dtypes.bfloat16)
        in_maps.append(m)
    return dict(npad=npad, nt=nt, kws=kws, n_real=n, newpos=newpos), in_maps


def run(in_maps, cfg, **kw):
    from concourse.bass_utils import run_bass_kernel_spmd

    key = (cfg["npad"], cfg["nt"], tuple(cfg["kws"]), cfg["n_real"])
    if key not in _NC_CACHE:
        _NC_CACHE[key] = build_program(*key)
    nc = _NC_CACHE[key]
    res = run_bass_kernel_spmd(nc, in_maps, core_ids=list(range(NCORES)), **kw)
    n = cfg["n_real"]
    nodes_pc = cfg["npad"] // NCORES
    parts = []
    for c in range(NCORES):
        o = np.asarray(res.results[c]["out"])
        pos_c = cfg["newpos"][c * nodes_pc:(c + 1) * nodes_pc]
        parts.append(o[pos_c])
    full = np.concatenate(parts, axis=0)[:n]
    return np.ascontiguousarray(full, dtype=np.float32), res


def kernel(x, src, dst, W, b, gamma, beta):
    cfg, in_maps = host_prep(x, src, dst, W, b, gamma, beta)
    out, _ = run(in_maps, cfg)
    return out


# revision 27
# speedup vs baseline: 1.2393x; 1.0133x over previous
"""DeepGCNLayer (GraphConv norm='both' + BatchNorm + ReLU + residual) on 8 trn2 cores.

Sharding: nodes padded to NPAD=100352, split into 8 ranges (98 node-tiles of
128 per core). Edges routed to the core owning their dst (dst-sorted), then
per (dst-tile, src-window) padded to a uniform K_w chunks of 128 so every
core runs one SPMD program.

v2 layout: the AllGather'd z table is built in 4 row-interleaved windows
(each window = the same quarter of every core's slice) so per-window
AllGathers overlap the z pass and the first gather groups. The one-hot S
matrices are built as ACT-engine broadcast expansion of eloc plus a DVE
is_equal on real tiles (2x perf mode, short shared-port holds) - the v1
broadcast tensor_tensor held the DVE shared SBUF port for ~28us/group,
starving the SWDGE gather descriptor generator (see trainium-docs
memories/01-sbuf.md "DVE blocks DMA" trap). x ships transposed bf16 so the
z pass needs no PE transposes; norm_src/norm_dst fold into ACT scale-copies.
"""

import sys

if "/opt/trn_rl_repo" not in sys.path:
    sys.path.insert(0, "/opt/trn_rl_repo")

import numpy as np

P = 128
D = 128
NCORES = 8
BN_EPS = 1e-5
GROUP = 7            # dst tiles per gather group
PBC = 7              # node tiles per phase-B load/store batch
WTILES = (22, 27, 27, 22)   # z-table window sizes in node tiles (per core)

_NC_CACHE = {}


def build_program(npad, nt, kws, n_real):
    """kws: tuple of chunks-per-window per dst tile (uniform across tiles)."""
    import concourse.bacc as bacc
    import concourse.tile as tile
    from concourse import mybir

    f32 = mybir.dt.float32
    bf16 = mybir.dt.bfloat16
    i32 = mybir.dt.int32
    i16 = mybir.dt.int16
    OP = mybir.AluOpType
    AF = mybir.ActivationFunctionType

    nodes_pc = nt * P
    ktot = sum(kws)
    nwin = len(kws)
    ngroups = nt // GROUP
    assert nt % GROUP == 0
    assert sum(WTILES) == nt and len(WTILES) == nwin
    kmax = max(kws)
    # idx16 columns per (group, window); eloc columns per (group, window)
    cols_gw = [GROUP * kw * P // 16 for kw in kws]
    gcols_i = sum(cols_gw)              # idx cols per group
    gcols_e = GROUP * ktot              # eloc cols per group

    nc = bacc.Bacc("TRN2", target_bir_lowering=False, debug=False,
                   num_devices=NCORES, num_swdge_queues=4)

    xt = nc.dram_tensor("xt", [P, nodes_pc], bf16, kind="ExternalInput")
    xres = nc.dram_tensor("xres", [nodes_pc, D], f32, kind="ExternalInput")
    wmat = nc.dram_tensor("wmat", [D, D], bf16, kind="ExternalInput")
    grow = nc.dram_tensor("grow", [1, D], f32, kind="ExternalInput")
    brow = nc.dram_tensor("brow", [1, D], f32, kind="ExternalInput")
    dgo = nc.dram_tensor("dgo", [P, nt], i32, kind="ExternalInput")
    dgi = nc.dram_tensor("dgi", [P, nt], i32, kind="ExternalInput")
    idxs = nc.dram_tensor("idxs", [P, ngroups * gcols_i], i16,
                          kind="ExternalInput")
    eloc = nc.dram_tensor("eloc", [P, ngroups * gcols_e], bf16,
                          kind="ExternalInput")
    out = nc.dram_tensor("out", [nodes_pc, D], f32, kind="ExternalOutput")
    h_d = nc.dram_tensor("h_d", [nodes_pc, D], bf16)

    # per-window z contribution + AllGather'd table (row-interleaved:
    # window w = [core0 quarter w | core1 quarter w | ...])
    cc_z = [nc.dram_tensor(f"cc_z{w}", [WTILES[w] * P, D], bf16)
            for w in range(nwin)]
    z_t = [nc.dram_tensor(f"z_t{w}", [WTILES[w] * P * NCORES, D], bf16,
                          addr_space="Shared")
           for w in range(nwin)]

    with tile.TileContext(nc) as tc:
        with (
            tc.tile_pool(name="const", bufs=1) as constp,
            tc.tile_pool(name="norm", bufs=1) as normp,
            tc.tile_pool(name="xz", bufs=1) as xzp,
            tc.tile_pool(name="zst", bufs=1) as zsp,
            tc.tile_pool(name="meta", bufs=2) as metap,
            tc.tile_pool(name="gathA", bufs=3) as gathA,
            tc.tile_pool(name="gathB", bufs=3) as gathB,
            tc.tile_pool(name="s", bufs=1) as sp,
            tc.tile_pool(name="work", bufs=2) as workp,
            tc.tile_pool(name="stats", bufs=1) as statp,
            tc.tile_pool(name="io", bufs=3) as iop,
            tc.tile_pool(name="psA", bufs=2, space="PSUM") as psA,
            tc.tile_pool(name="psB", bufs=2, space="PSUM") as psB,
            tc.tile_pool(name="psS", bufs=1, space="PSUM") as psS,
            tc.tile_pool(name="dram", bufs=2, space="DRAM") as dramp,
        ):
            # ---- constants -------------------------------------------------
            iota = constp.tile([P, P], bf16, tag="iota")
            nc.gpsimd.iota(iota[:], pattern=[[1, P]], base=0,
                           channel_multiplier=0,
                           allow_small_or_imprecise_dtypes=True)
            # iota replicated along free dim for batched 2x is_equal
            iota_rep = constp.tile([P, GROUP * kmax * P], bf16, tag="iotar")
            nc.vector.tensor_copy(
                iota_rep[:].rearrange("p (c e) -> p c e", e=P),
                iota[:, None, :].to_broadcast([P, GROUP * kmax, P]))
            ones1 = constp.tile([1, P], f32, tag="ones1")
            nc.vector.memset(ones1[:], 1.0)
            ones_c = constp.tile([P, 1], bf16, tag="ones_c")
            nc.vector.memset(ones_c[:], 1.0)
            w_sb = constp.tile([P, D], bf16, tag="wsb")
            nc.sync.dma_start(out=w_sb[:], in_=wmat[:])
            g_row = constp.tile([1, D], f32, tag="grow")
            nc.sync.dma_start(out=g_row[:], in_=grow[:])
            be_row = constp.tile([1, D], f32, tag="berow")
            nc.sync.dma_start(out=be_row[:], in_=brow[:])

            # ---- norm arrays (own range, F-order [P, nt]) ------------------
            deg = normp.tile([P, nt], i32, tag="deg")
            nc.sync.dma_start(out=deg[:], in_=dgo[:])
            degf = normp.tile([P, nt], f32, tag="degf")
            nc.vector.tensor_scalar_max(degf[:], deg[:], 1.0)
            nc.scalar.sqrt(degf[:], degf[:])
            ns_f = constp.tile([P, nt], f32, tag="ns_f")
            nc.vector.reciprocal(ns_f[:], degf[:])

            deg2 = normp.tile([P, nt], i32, tag="deg2")
            nc.sync.dma_start(out=deg2[:], in_=dgi[:])
            deg2f = normp.tile([P, nt], f32, tag="deg2f")
            nc.vector.tensor_scalar_max(deg2f[:], deg2[:], 1.0)
            nc.scalar.sqrt(deg2f[:], deg2f[:])
            nd_f = constp.tile([P, nt], f32, tag="nd_f")
            nc.vector.reciprocal(nd_f[:], deg2f[:])

            # ---- z pass: z = (x@W)*ns -> bf16, own slice, 4 windows --------
            # lhsT = xT slice (no transposes); ns folds into the ACT
            # PSUM->SBUF copy; each window's AllGather issues as soon as its
            # quarter is stored so window-0 gathers start early.
            t0 = 0
            for w in range(nwin):
                wt = WTILES[w]
                xt_w = xzp.tile([P, wt * P], bf16, tag="xt_w")
                nc.sync.dma_start(out=xt_w[:],
                                  in_=xt[:, t0 * P:(t0 + wt) * P])
                z_w = zsp.tile([P, wt * D], bf16, tag="z_w")
                for c in range(wt):
                    z_ps = psB.tile([P, D], f32, tag="B")
                    nc.tensor.matmul(out=z_ps[:],
                                     lhsT=xt_w[:, c * P:(c + 1) * P],
                                     rhs=w_sb[:], start=True, stop=True)
                    nc.scalar.activation(
                        out=z_w[:, c * D:(c + 1) * D], in_=z_ps[:],
                        func=AF.Copy, scale=ns_f[:, t0 + c:t0 + c + 1])
                nc.scalar.dma_start(
                    out=cc_z[w][:].rearrange("(c p) e -> p c e", p=P),
                    in_=z_w[:].rearrange("p (c e) -> p c e", e=D))
                nc.gpsimd.collective_compute(
                    "AllGather", OP.bypass,
                    replica_groups=[list(range(NCORES))],
                    ins=[cc_z[w][:]], outs=[z_t[w][:]])
                t0 += wt

            # ---- phase A ---------------------------------------------------
            sum_ps = psS.tile([1, P], f32, tag="sum")
            sq_ps = psS.tile([1, P], f32, tag="sq")

            for g in range(ngroups):
                idx_g = metap.tile([P, gcols_i], i16, tag="idxg")
                nc.sync.dma_start(
                    out=idx_g[:], in_=idxs[:, g * gcols_i:(g + 1) * gcols_i])
                eloc_g = metap.tile([P, gcols_e], bf16, tag="elocg")
                nc.sync.dma_start(
                    out=eloc_g[:], in_=eloc[:, g * gcols_e:(g + 1) * gcols_e])

                e_ws = []
                s_ws = []
                ico = 0
                eco = 0
                for w in range(nwin):
                    kw = kws[w]
                    nch = GROUP * kw
                    nidx = nch * P
                    pool = gathA if w < 2 else gathB
                    e_t = pool.tile([P, nch * D], bf16, tag=f"E{w}")
                    nc.gpsimd.dma_gather(
                        e_t[:].rearrange("p (c e) -> p c e", e=D),
                        z_t[w][:],
                        idx_g[:, ico:ico + nidx // 16],
                        nidx, nidx, D, single_packet=False,
                        queue_num=(g + w) % 4)
                    ico += nidx // 16
                    e_ws.append(e_t)
                    s_t = sp.tile([P, nch * P], bf16, tag=f"S{w}")
                    nc.vector.tensor_tensor(
                        out=s_t[:].rearrange("p (c e) -> p c e", e=P),
                        in0=eloc_g[:, eco:eco + nch, None].to_broadcast(
                            [P, nch, P]),
                        in1=iota_rep[:, :nch * P].rearrange(
                            "p (c e) -> p c e", e=P),
                        op=OP.is_equal)
                    eco += nch
                    s_ws.append(s_t)

                h_g = workp.tile([P, GROUP * D], bf16, tag="hg")
                for u in range(GROUP):
                    t = g * GROUP + u
                    agg_ps = psA.tile([P, P], f32, tag="A")
                    ci = 0
                    for w in range(nwin):
                        kw = kws[w]
                        for j in range(kw):
                            nc.tensor.matmul(
                                out=agg_ps[:],
                                lhsT=s_ws[w][:, (u * kw + j) * P:
                                             (u * kw + j + 1) * P],
                                rhs=e_ws[w][:, (u * kw + j) * D:
                                            (u * kw + j + 1) * D],
                                start=(ci + j == 0),
                                stop=(ci + j == ktot - 1))
                        ci += kw
                    h_t = h_g[:, u * D:(u + 1) * D]
                    nc.scalar.activation(out=h_t, in_=agg_ps[:],
                                         func=AF.Copy,
                                         scale=nd_f[:, t:t + 1])
                    sq_sb = workp.tile([P, D], bf16, tag="sqsb")
                    nc.scalar.activation(out=sq_sb[:], in_=h_t,
                                         func=AF.Square)
                    nc.tensor.matmul(out=sum_ps[:], lhsT=ones_c[:], rhs=h_t,
                                     start=(t == 0), stop=(t == nt - 1))
                    nc.tensor.matmul(out=sq_ps[:], lhsT=ones_c[:],
                                     rhs=sq_sb[:],
                                     start=(t == 0), stop=(t == nt - 1))
                nc.scalar.dma_start(
                    out=h_d[:].rearrange("(c p) e -> p c e", p=P)[
                        :, g * GROUP:(g + 1) * GROUP, :],
                    in_=h_g[:].rearrange("p (c e) -> p c e", e=D))

            # ---- BN stats all-reduce + scale/shift (row layout) ------------
            srow = statp.tile([1, 2 * P], f32, tag="srow")
            nc.scalar.copy(srow[0:1, 0:P], sum_ps[:])
            nc.scalar.copy(srow[0:1, P:2 * P], sq_ps[:])
            cc_in = dramp.tile([1, 2 * P], f32, tag="ccin")
            cc_out = dramp.tile([1, 2 * P], f32, tag="ccout")
            nc.gpsimd.dma_start(out=cc_in[:], in_=srow[:])
            nc.gpsimd.collective_compute(
                "AllReduce", OP.add,
                replica_groups=[list(range(NCORES))],
                ins=[cc_in.opt()], outs=[cc_out.opt()])
            grow_sb = statp.tile([1, 2 * P], f32, tag="grow_sb")
            nc.gpsimd.dma_start(out=grow_sb[:], in_=cc_out[:])

            inv_n = 1.0 / float(n_real)
            mean_r = statp.tile([1, P], f32, tag="mean")
            nc.vector.tensor_scalar_mul(mean_r[:], grow_sb[0:1, 0:P], inv_n)
            ex2_r = statp.tile([1, P], f32, tag="ex2")
            nc.vector.tensor_scalar_mul(ex2_r[:], grow_sb[0:1, P:2 * P],
                                        inv_n)
            m2_r = statp.tile([1, P], f32, tag="m2")
            nc.scalar.square(m2_r[:], mean_r[:])
            var_r = statp.tile([1, P], f32, tag="var")
            nc.vector.tensor_tensor(out=var_r[:], in0=ex2_r[:], in1=m2_r[:],
                                    op=OP.subtract)
            nc.vector.tensor_scalar_add(var_r[:], var_r[:], BN_EPS)
            sd_r = statp.tile([1, P], f32, tag="sd")
            nc.scalar.sqrt(sd_r[:], var_r[:])
            inv_r = statp.tile([1, P], f32, tag="inv")
            nc.vector.reciprocal(inv_r[:], sd_r[:])
            sc_r = statp.tile([1, P], f32, tag="sc")
            nc.vector.tensor_tensor(out=sc_r[:], in0=g_row[:], in1=inv_r[:],
                                    op=OP.mult)
            # b cancels in BN: shift = beta - mean*scale
            tc_r = statp.tile([1, P], f32, tag="tc")
            nc.vector.tensor_tensor(out=tc_r[:], in0=mean_r[:], in1=sc_r[:],
                                    op=OP.mult)
            nc.vector.tensor_tensor(out=tc_r[:], in0=be_row[:], in1=tc_r[:],
                                    op=OP.subtract)

            # rank-1 broadcast of sc/tc across partitions -> bf16 tiles
            scb_ps = psA.tile([P, P], f32, tag="A")
            nc.tensor.matmul(out=scb_ps[:], lhsT=ones1[:], rhs=sc_r[:],
                             start=True, stop=True)
            sc_bc = constp.tile([P, P], bf16, tag="sc_bc")
            nc.scalar.copy(sc_bc[:], scb_ps[:])
            tcb_ps = psB.tile([P, P], f32, tag="B")
            nc.tensor.matmul(out=tcb_ps[:], lhsT=ones1[:], rhs=tc_r[:],
                             start=True, stop=True)
            tc_bc = constp.tile([P, P], bf16, tag="tc_bc")
            nc.scalar.copy(tc_bc[:], tcb_ps[:])

            # ---- phase B (batched loads/stores, no transposes) -------------
            for bt in range(nt // PBC):
                t0 = bt * PBC
                x_b = iop.tile([P, PBC * D], f32, tag="xb")
                nc.scalar.dma_start(
                    out=x_b[:].rearrange("p (c e) -> p c e", e=D),
                    in_=xres[:].rearrange("(c p) e -> p c e", p=P)[
                        :, t0:t0 + PBC, :])
                h_b = iop.tile([P, PBC * D], bf16, tag="hb")
                nc.sync.dma_start(
                    out=h_b[:].rearrange("p (c e) -> p c e", e=D),
                    in_=h_d[:].rearrange("(c p) e -> p c e", p=P)[
                        :, t0:t0 + PBC, :])
                g1 = workp.tile([P, PBC * D], bf16, tag="g1")
                nc.vector.tensor_tensor(
                    out=g1[:].rearrange("p (c e) -> p c e", e=D),
                    in0=h_b[:].rearrange("p (c e) -> p c e", e=D),
                    in1=sc_bc[:, None, :].to_broadcast([P, PBC, D]),
                    op=OP.mult)
                nc.vector.tensor_tensor(
                    out=g1[:].rearrange("p (c e) -> p c e", e=D),
                    in0=g1[:].rearrange("p (c e) -> p c e", e=D),
                    in1=tc_bc[:, None, :].to_broadcast([P, PBC, D]),
                    op=OP.add)
                nc.vector.tensor_scalar_max(g1[:], g1[:], 0.0)
                nc.vector.tensor_tensor(out=x_b[:], in0=g1[:], in1=x_b[:],
                                        op=OP.add)
                nc.sync.dma_start(
                    out=out[:].rearrange("(c p) e -> p c e", p=P)[
                        :, t0:t0 + PBC, :],
                    in_=x_b[:].rearrange("p (c e) -> p c e", e=D))

    nc.compile()
    return nc


def _wrap16(a):
    b = a.reshape(-1, 16).T
    return np.tile(b, (8, 1))


def host_prep(x, src, dst, W, b, gamma, beta):
    """Graph routing / layout prep (indices only - no FLOPs on host)."""
    import ml_dtypes

    x = np.asarray(x, np.float32)
    W = np.asarray(W, np.float32)
    gamma = np.asarray(gamma, np.float32)
    beta = np.asarray(beta, np.float32)
    src32 = np.asarray(src).astype(np.int64)
    dst32 = np.asarray(dst).astype(np.int64)

    n = x.shape[0]
    npad = -(-n // (P * NCORES * GROUP)) * (P * NCORES * GROUP)
    nodes_pc = npad // NCORES
    nt = nodes_pc // P
    nt_tot = npad // P
    assert sum(WTILES) == nt
    nwin = len(WTILES)
    wt_start = np.cumsum([0] + list(WTILES))  # in tiles, per core

    order = np.argsort(dst32, kind="stable")
    ds = dst32[order]
    ss = src32[order]

    ar = np.arange(npad + 1, dtype=np.int64)
    rps = np.searchsorted(np.sort(src32), ar).astype(np.int32)
    rpd_full = np.searchsorted(ds, ar)

    # src -> (window, row within window table). Window w of the z table is
    # [core0 quarter w | core1 quarter w | ...], quarter w = tiles
    # [wt_start[w], wt_start[w+1]) of each core's slice.
    s_core = ss // nodes_pc
    s_r = ss % nodes_pc
    s_tile = s_r // P
    s_win = np.searchsorted(wt_start, s_tile, side="right") - 1
    wrows = (np.array(WTILES) * P)[s_win]
    s_winrow = s_core * wrows + (s_r - wt_start[s_win] * P)

    # degree counts (int), F-order [P, nt] per core
    dgo_n = np.diff(rps).astype(np.int32)                 # [npad]
    dgi_n = np.diff(rpd_full).astype(np.int32)            # [npad]

    # per-dst in-degree split by src window
    deg4 = np.zeros((npad, nwin), np.int32)
    np.add.at(deg4, (ds, s_win), 1)

    # --- bin-pack dst nodes into tiles (per core) to flatten the
    # per-(tile, window) edge-count tails, so kws (chunk counts) shrink.
    caps = np.array([4 * P, 5 * P, 5 * P, 4 * P], np.float64)
    newpos = np.empty(npad, np.int64)    # global node -> permuted local slot
    for c in range(NCORES):
        d4 = deg4[c * nodes_pc:(c + 1) * nodes_pc].astype(np.float64)
        order_c = np.argsort(-d4.sum(1), kind="stable")
        loads = np.zeros((nt, nwin))
        counts = np.zeros(nt, np.int64)
        fill = [[] for _ in range(nt)]
        for i in order_c:
            util = np.max((loads + d4[i]) / caps, axis=1)
            util[counts >= P] = np.inf
            b = int(np.argmin(util))
            loads[b] += d4[i]
            counts[b] += 1
            fill[b].append(i)
        pos = np.empty(nodes_pc, np.int64)
        for b in range(nt):
            pos[np.array(fill[b], np.int64)] = (
                b * P + np.arange(len(fill[b])))
        newpos[c * nodes_pc:(c + 1) * nodes_pc] = pos

    # kws from the packed loads (global max over cores/tiles per window)
    e_core = ds // nodes_pc
    e_pos = newpos[ds]
    e_tile = e_core * nt + e_pos // P
    e_slot = e_pos % P
    cell = e_tile * nwin + s_win
    cnt = np.bincount(cell, minlength=nt_tot * nwin).reshape(nt_tot, nwin)
    kws = tuple(int(-(-cnt[:, w].max() // P)) for w in range(nwin))
    ktot = sum(kws)

    # per (tile, window) edge lists under the permutation
    eorder = np.argsort(cell, kind="stable")
    bnd = np.searchsorted(cell[eorder], np.arange(nt_tot * nwin + 1))
    tw_lists = [[eorder[bnd[t * nwin + w]:bnd[t * nwin + w + 1]]
                 for w in range(nwin)] for t in range(nt_tot)]

    xpad = np.zeros((npad, D), np.float32)
    xpad[:n] = x

    ngroups = nt // GROUP
    in_maps = []
    shared = dict(
        wmat=W.astype(ml_dtypes.bfloat16),
        grow=np.ascontiguousarray(gamma[None, :]),
        brow=np.ascontiguousarray(beta[None, :]))
    for c in range(NCORES):
        # eloc layout: (g, w, u, chunk) contiguous for batched expansions
        elocv = np.full((nt * ktot, P), -1.0, np.float32)
        idx_blocks = []
        ecol_off = 0
        for g in range(ngroups):
            for w in range(nwin):
                blk = np.zeros(GROUP * kws[w] * P, np.int16)
                for u in range(GROUP):
                    t = g * GROUP + u
                    gt = c * nt + t
                    sel = tw_lists[gt][w]
                    base = u * kws[w] * P
                    blk[base:base + len(sel)] = s_winrow[sel].astype(np.int16)
                    ev = e_slot[sel].astype(np.float32)
                    ecol = elocv[ecol_off + u * kws[w]:
                                 ecol_off + (u + 1) * kws[w]].reshape(-1)
                    ecol[:len(sel)] = ev
                ecol_off += GROUP * kws[w]
                idx_blocks.append(_wrap16(blk))
        m = dict(shared)
        xslice = xpad[c * nodes_pc:(c + 1) * nodes_pc]
        pos_c = newpos[c * nodes_pc:(c + 1) * nodes_pc]
        orig_of = np.empty(nodes_pc, np.int64)
        orig_of[pos_c] = np.arange(nodes_pc)
        m["xt"] = np.ascontiguousarray(xslice.T).astype(ml_dtypes.bfloat16)
        m["xres"] = np.ascontiguousarray(xslice[orig_of])
        m["dgo"] = np.ascontiguousarray(
            dgo_n[c * nodes_pc:(c + 1) * nodes_pc].reshape(nt, P).T)
        m["dgi"] = np.ascontiguousarray(
            dgi_n[c * nodes_pc:(c + 1) * nodes_pc][orig_of]
            .reshape(nt, P).T)
        m["idxs"] = np.ascontiguousarray(np.concatenate(idx_blocks, axis=1))
        # eloc device layout: col (g,w,u,chunk) partition p = edge c*128+p
        m["eloc"] = np.ascontiguousarray(
            elocv.T).astype(ml_dtypes.bfloat16)
        in_maps.append(m)
    return dict(npad=npad, nt=nt, kws=kws, n_real=n, newpos=newpos), in_maps


def run(in_maps, cfg, **kw):
    from concourse.bass_utils import run_bass_kernel_spmd

    key = (cfg["npad"], cfg["nt"], tuple(cfg["kws"]), cfg["n_real"])
    if key not in _NC_CACHE:
        _NC_CACHE[key] = build_program(*key)
    nc = _NC_CACHE[key]
    res = run_bass_kernel_spmd(nc, in_maps, core_ids=list(range(NCORES)), **kw)
    n = cfg["n_real"]
    nodes_pc = cfg["npad"] // NCORES
    parts = []
    for c in range(NCORES):
        o = np.asarray(res.results[c]["out"])
        pos_c = cfg["newpos"][c * nodes_pc:(c + 1) * nodes_pc]
        parts.append(o[pos_c])
    full = np.concatenate(parts, axis=0)[:n]
    return np.ascontiguousarray(full, dtype=np.float32), res


def kernel(x, src, dst, W, b, gamma, beta):
    cfg, in_maps = host_prep(x, src, dst, W, b, gamma, beta)
    out, _ = run(in_maps, cfg)
    return out
